# revision 1
# baseline (speedup 1.0000x reference)
"""CNLinkPredictor Trainium2 kernel.

Edge-sharded across 8 NeuronCores (1024 target edges each); x, adj, and the
MLP weights are replicated. Per core:
  A) h = x + MLP(x) computed in transposed layout: the host supplies xT, so
     stage A is matmul-only on PE (bf16, N=512 moving), fused bias+ReLU on
     the scalar engine, residual on DVE, then xbar DMA-transposes write h
     back to natural layout (bf16, (half, ktile, c) column order so every
     transpose destination is a contiguous per-partition span).
  B) per 128-edge block and k-half: indirect-DMA gather of the two adjacency
     rows per edge (fp8 - exact for a 0/1 adjacency - one row per SBUF
     partition), DVE multiply -> cn (bf16, still exact), one xbar
     DMA-transpose, then 32 matmuls accumulating cnT @ h into PSUM.
  C) edge MLPs in transposed layout (bf16, N=512 over 4-block groups), xbar
     transposes for xcn and xi*xj, final [1, 1024] output row.

Emission is software-pipelined (A first half, B k-half 0, A second half,
B k-half 1, C, ...) so the FIFO engine queues never head-of-line block on
data that is not ready yet.

Hardware pitfalls this kernel works around:
  - This walrus build accepts at most ONE sync-wait per instruction
    (_apply_tile_patch splits the Tile tail drain; _split_multi_waits hoists
    extra waits onto same-engine NoOps).
  - Concurrent 4-byte DMA traffic corrupts in-flight 2-byte xbar
    DMA-transposes, so every steady-state transfer is <= 2 bytes/element
    (fp8 adjacency, bf16 everything else); the few f32/int32 loads happen
    up front and the single f32 store happens after the last transpose.
  - xbar transposes into non-contiguous destinations produce wrong data;
    all transpose targets collapse to contiguous 2-D access patterns.
"""

import numpy as np
import ml_dtypes

N = 8192
C = 256
E = 8192
NCORES = 8
EL = E // NCORES          # edges per core
P = 128
NB = EL // P              # edge blocks per core
KH = 2                    # k halves for adjacency gather
KC = N // KH              # columns per half
NKT = N // P              # 64 k tiles
AGRP = 512                # stage-A node group
CGRP = 4                  # stage-C blocks per group (512 edges)

_CACHE = {}
TRACE = False
LAST_RESULT = None
DEBUG_DUMPS = False


def _apply_tile_patch():
    """Split the Tile tail-drain's multi-sem wait onto individual SP nops."""
    from concourse.tile import TileContext
    from concourse.vector_clock import ScopedClock

    if getattr(TileContext, "_drain_patched", False):
        return

    def _patched(self, tick_clock, wait_clock):
        nc = self.nc
        collector = nc.sync.nop()
        wait_clock.add_sem_waits(
            collector.ins, ScopedClock({None: tick_clock.global_clock})
        )
        si = collector.ins.sync_info
        waits = list(si.on_wait) if si is not None and si.on_wait else []
        if si is not None and len(waits) > 1:
            name_to_handle = {h.name: h for h in self.sems.allocated().values()}
            si.on_wait = [waits[0]]
            for w in waits[1:]:
                op = {
                    "sem-ge-imm": "sem-ge",
                    "sem-eq-imm": "sem-eq",
                    "sem-le-imm": "sem-le",
                }.get(str(w.wait_mode), "sem-ge")
                nc.sync.nop().wait_op(name_to_handle[w.ant_name], w.wait_value, op)
        nc.sync.drain()
        nc.all_engine_barrier()
        assert self.sems is not None
        popped = nc._tile_sem_poison_stack.pop()
        assert popped is self._sem_poison
        nc.clear_and_free_semaphores(list(self.sems.allocated().values()))
        nc.all_engine_barrier()

    TileContext._drain_and_barrier = _patched
    TileContext._drain_patched = True


def _split_multi_waits(nc):
    """Hoist extra sync-waits onto same-engine NoOps (sequential waits ==
    ANDed waits); this walrus build allows one wait per instruction."""
    import concourse.mybir as mybir

    cnt = 0
    for fn in nc.m.functions:
        for bb in fn.blocks:
            out = []
            for inst in bb.instructions:
                si = getattr(inst, "sync_info", None)
                waits = list(si.on_wait) if si is not None and si.on_wait else []
                if len(waits) > 1:
                    for w in waits[:-1]:
                        nop = mybir.InstNoOp(name=f"ws-{cnt}", ins=[], outs=[])
                        cnt += 1
                        nop.engine = inst.engine
                        nop.sync_info = mybir.SyncInfo(on_wait=[w], on_update=[])
                        out.append(nop)
                    si.on_wait = [waits[-1]]
                out.append(inst)
            bb.instructions = out
    return nc


def _build(split_waits=True):
    import concourse.bass as bass
    import concourse.mybir as mybir
    from concourse.tile import TileContext

    _apply_tile_patch()

    f32 = mybir.dt.float32
    f32r = mybir.dt.float32r
    bf16 = mybir.dt.bfloat16
    fp8 = mybir.dt.float8e4
    i32 = mybir.dt.int32
    Relu = mybir.ActivationFunctionType.Relu
    Ident = mybir.ActivationFunctionType.Identity
    MUL = mybir.AluOpType.mult
    ADD = mybir.AluOpType.add

    nc = bass.Bass(num_swdge_queues=4)

    xT_d = nc.dram_tensor("xT", [C, N], bf16, kind="ExternalInput")
    x_d = nc.dram_tensor("x", [N, C], bf16, kind="ExternalInput")
    adj_d = nc.dram_tensor("adj", [N, N], fp8, kind="ExternalInput")
    idx_d = nc.dram_tensor("idx", [2, EL], i32, kind="ExternalInput")
    # all matmul weights in bf16 (2-byte rule; see module docstring)
    wA = {n: nc.dram_tensor(n, [C, C], bf16, kind="ExternalInput")
          for n in ("xlin_w1", "xlin_w2")}
    wC = {n: nc.dram_tensor(n, [C, C], bf16, kind="ExternalInput")
          for n in ("xcn_w1", "xcn_w2", "xij_w", "lin_w1")}
    lin_w2_d = nc.dram_tensor("lin_w2", [C, 1], bf16, kind="ExternalInput")
    bnames = ["xlin_b1", "xlin_b2", "xcn_b1", "xcn_b2", "xij_b", "lin_b1"]
    ball_d = nc.dram_tensor("ball", [P, 2 * len(bnames)], f32,
                            kind="ExternalInput")
    lin_b2_d = nc.dram_tensor("lin_b2", [1, 1], f32, kind="ExternalInput")
    beta_d = nc.dram_tensor("beta_bc", [P, 1], f32, kind="ExternalInput")
    out_d = nc.dram_tensor("out", [1, EL], f32, kind="ExternalOutput")
    dbg = {}
    if DEBUG_DUMPS:
        dbg["h_all"] = nc.dram_tensor("dbg_h", [P, 2 * N], bf16,
                                      kind="ExternalOutput")
        dbg["cn"] = nc.dram_tensor("dbg_cn", [P, KC], bf16,
                                   kind="ExternalOutput")
        dbg["cnT"] = nc.dram_tensor("dbg_cnT", [P, KC], bf16,
                                    kind="ExternalOutput")
        dbg["xcn"] = nc.dram_tensor("dbg_xcn", [P, C], bf16,
                                    kind="ExternalOutput")
        dbg["xcnT"] = nc.dram_tensor("dbg_xcnT", [P, 2 * CGRP * P], bf16,
                                     kind="ExternalOutput")
        dbg["prodT"] = nc.dram_tensor("dbg_prodT", [P, 2 * CGRP * P], bf16,
                                      kind="ExternalOutput")

    _swq = [0]

    def _rr(inst):
        q = _swq[0] % 4
        _swq[0] += 1
        if q:
            inst.ins.queue = f"qPoolDynamic{q}"
        return inst

    with TileContext(nc) as tc:
        with (
            tc.tile_pool(name="const", bufs=1) as pK,
            tc.tile_pool(name="hpool", bufs=1) as pH,
            tc.tile_pool(name="adj", bufs=5) as pAdj,
            tc.tile_pool(name="cn", bufs=4) as pCn,
            tc.tile_pool(name="cnT", bufs=4) as pT,
            tc.tile_pool(name="edge", bufs=2) as pC,
            tc.tile_pool(name="xcn", bufs=CGRP) as pX,
        ):
            # ---- constants ----
            # idx first: the stage-B gathers depend only on these
            idx_sb = pK.tile([P, 2 * NB], i32, tag="idx_sb", name="idx_sb")
            nc.sync.dma_start(
                out=idx_sb[:].rearrange("p (t b) -> p t b", t=2),
                in_=idx_d[:, :].rearrange("t (b p) -> p t b", p=P),
            )
            ii = [idx_sb[:, b:b + 1] for b in range(NB)]
            jj = [idx_sb[:, NB + b:NB + b + 1] for b in range(NB)]

            wA_sb, wC_sb = {}, {}
            for n, t_d in list(wA.items()) + list(wC.items()):
                t = pK.tile([P, 2 * C], bf16, tag=f"w_{n}", name=f"w_{n}")
                nc.sync.dma_start(
                    out=t[:].rearrange("p (k n2) -> p k n2", k=2),
                    in_=t_d[:, :].rearrange("(k p) n2 -> p k n2", p=P),
                )
                pair = [t[:, 0:C], t[:, C:2 * C]]
                (wA_sb if n in wA else wC_sb)[n] = pair
            lw2_t = pK.tile([P, 2], bf16, tag="lin_w2", name="lin_w2t")
            nc.sync.dma_start(
                out=lw2_t[:].rearrange("p (k o) -> p k o", k=2),
                in_=lin_w2_d[:, :].rearrange("(k p) o -> p k o", p=P),
            )
            lw2_sb = [lw2_t[:, 0:1], lw2_t[:, 1:2]]
            b_sb = {}
            ball = pK.tile([P, 2 * len(bnames)], f32, tag="ball", name="ball")
            nc.sync.dma_start(
                out=ball[:],
                in_=ball_d[:, :],
            )
            for q, n in enumerate(bnames):
                b_sb[n] = ball[:, 2 * q:2 * q + 2]
            lb2_sb = pK.tile([1, 1], f32, tag="b_lin2", name="b_lin2")
            nc.sync.dma_start(out=lb2_sb[:], in_=lin_b2_d[:, :])
            beta_sb = pK.tile([P, 1], f32, tag="beta", name="beta")
            nc.sync.dma_start(out=beta_sb[:], in_=beta_d[:, :])

            out_row = pK.tile([1, EL], f32, tag="out_row", name="out_row")
            # natural-layout h in (hh, kt, c2) order so the xbar transposes
            # write contiguous per-partition spans: column = hh*N + kt*128 + c2
            # encodes h[node = kt*128 + p, channel = hh*128 + c2].
            h_all = pH.tile([P, 2 * N], bf16, tag="h_all", name="h_all")
            h_view = h_all[:].rearrange("p (hh kt c) -> p hh kt c", hh=2, c=P)

            # ---- stage definitions ----
            def stage_a_group(g, pA, psA):
                m0 = g * AGRP
                xT = []
                for h in range(2):
                    t = pA.tile([P, AGRP], bf16, tag=f"xT{h}", name=f"xT{h}_{g}")
                    nc.scalar.dma_start(
                        out=t[:], in_=xT_d[h * P:(h + 1) * P, m0:m0 + AGRP]
                    )
                    xT.append(t)
                y1T = []
                for h in range(2):
                    ps = psA.tile([P, AGRP], f32, tag="psmm", name=f"psA1_{g}{h}")
                    nc.tensor.matmul(
                        ps[:], wA_sb["xlin_w1"][0][:, h * P:(h + 1) * P],
                        xT[0][:], start=True, stop=False,
                    )
                    nc.tensor.matmul(
                        ps[:], wA_sb["xlin_w1"][1][:, h * P:(h + 1) * P],
                        xT[1][:], start=False, stop=True,
                    )
                    t = pA.tile([P, AGRP], bf16, tag=f"y1T{h}", name=f"y1T{h}_{g}")
                    nc.scalar.activation(
                        t[:], ps[:], Relu, bias=b_sb["xlin_b1"][:, h:h + 1]
                    )
                    y1T.append(t)
                for h in range(2):
                    ps = psA.tile([P, AGRP], f32, tag="psmm", name=f"psA2_{g}{h}")
                    nc.tensor.matmul(
                        ps[:], wA_sb["xlin_w2"][0][:, h * P:(h + 1) * P],
                        y1T[0][:], start=True, stop=False,
                    )
                    nc.tensor.matmul(
                        ps[:], wA_sb["xlin_w2"][1][:, h * P:(h + 1) * P],
                        y1T[1][:], start=False, stop=True,
                    )
                    y2 = pA.tile([P, AGRP], bf16, tag="y2T", name=f"y2T{h}_{g}")
                    nc.scalar.activation(
                        y2[:], ps[:], Relu, bias=b_sb["xlin_b2"][:, h:h + 1]
                    )
                    hT = pA.tile([P, AGRP], bf16, tag=f"hT{h}", name=f"hT{h}_{g}")
                    nc.vector.tensor_tensor(
                        out=hT[:], in0=xT[h][:], in1=y2[:], op=ADD
                    )
                    nc.sync.dma_start_transpose(
                        out=h_view[:, h,
                                   g * (AGRP // P):(g + 1) * (AGRP // P), :],
                        in_=hT[:],
                    )

            xcn_sb = [None] * NB

            cnT_map = {}

            def stage_b_load(b, s):
                ai = pAdj.tile([P, KC], fp8, tag="ai", name=f"ai{b}_{s}")
                _rr(nc.gpsimd.indirect_dma_start(
                    out=ai[:], out_offset=None, in_=adj_d[:, :],
                    in_offset=bass.IndirectOffsetOnAxis(ap=ii[b][:, :1], axis=0),
                    element_offset=s * KC,
                ))
                aj = pAdj.tile([P, KC], fp8, tag="aj", name=f"aj{b}_{s}")
                _rr(nc.gpsimd.indirect_dma_start(
                    out=aj[:], out_offset=None, in_=adj_d[:, :],
                    in_offset=bass.IndirectOffsetOnAxis(ap=jj[b][:, :1], axis=0),
                    element_offset=s * KC,
                ))
                cn = pCn.tile([P, KC], bf16, tag="cn", name=f"cn{b}_{s}")
                nc.vector.tensor_tensor(out=cn[:], in0=ai[:], in1=aj[:], op=MUL)
                cnT = pT.tile([P, KC], bf16, tag="cnT", name=f"cnT{b}_{s}")
                nc.sync.dma_start_transpose(
                    out=cnT[:].rearrange("p (kt e) -> p kt e", e=P),
                    in_=cn[:],
                )
                cnT_map[(b, s)] = cnT

            def stage_b_mms(b, s, psxcn):
                cnT = cnT_map[(b, s)]
                for kt in range(KC // P):
                    ktg = s * (KC // P) + kt
                    nc.tensor.matmul(
                        psxcn[:],
                        cnT[:, kt * P:(kt + 1) * P],
                        h_view[:, :, ktg, :],
                        start=(ktg == 0), stop=(ktg == NKT - 1),
                    )

            def stage_b_finish(b, psxcn):
                xcn_sb[b] = pX.tile([P, C], bf16, tag="xcn", name=f"xcn{b}")
                nc.vector.tensor_copy(xcn_sb[b][:], psxcn[:])

            prodT_map = {}

            def stage_c_prod(grp):
                blocks = range(grp * CGRP, (grp + 1) * CGRP)
                W = CGRP * P
                prodT = pC.tile([P, 2 * W], bf16, tag="prodT", name=f"prodT{grp}")
                prodT_v = prodT[:].rearrange(
                    "p (blk hh e) -> p blk hh e", blk=CGRP, e=P)
                prodT_map[grp] = prodT
                for t2, b in enumerate(blocks):
                    xi = pC.tile([P, C], bf16, tag="xi", name=f"xi{b}")
                    _rr(nc.gpsimd.indirect_dma_start(
                        out=xi[:], out_offset=None, in_=x_d[:, :],
                        in_offset=bass.IndirectOffsetOnAxis(
                            ap=ii[b][:, :1], axis=0),
                    ))
                    xj = pC.tile([P, C], bf16, tag="xj", name=f"xj{b}")
                    _rr(nc.gpsimd.indirect_dma_start(
                        out=xj[:], out_offset=None, in_=x_d[:, :],
                        in_offset=bass.IndirectOffsetOnAxis(
                            ap=jj[b][:, :1], axis=0),
                    ))
                    pt = pC.tile([P, C], bf16, tag="prod", name=f"prod{b}")
                    nc.vector.tensor_tensor(
                        out=pt[:], in0=xi[:], in1=xj[:], op=MUL
                    )
                    nc.sync.dma_start_transpose(
                        out=prodT_v[:, t2, :, :], in_=pt[:],
                    )

            def stage_c(grp, psC, psO):
                blocks = range(grp * CGRP, (grp + 1) * CGRP)
                W = CGRP * P  # 512 edges
                xcnT = pC.tile([P, 2 * W], bf16, tag="xcnT", name=f"xcnT{grp}")
                xcnT_v = xcnT[:].rearrange(
                    "p (blk hh e) -> p blk hh e", blk=CGRP, e=P)
                prodT = prodT_map[grp]
                for t2, b in enumerate(blocks):
                    nc.sync.dma_start_transpose(
                        out=xcnT_v[:, t2, :, :], in_=xcn_sb[b][:],
                    )

                def mlp_layer(rhs2, wname, bname, outtag, packed):
                    outs = []
                    for h in range(2):
                        ps = psC.tile([P, W], f32, tag="psc",
                                      name=f"psc_{grp}_{outtag}{h}")
                        if packed:
                            rhs_v = rhs2[:].rearrange(
                                "p (blk hh e) -> p blk hh e", blk=CGRP, e=P)
                            r0, r1 = rhs_v[:, :, 0, :], rhs_v[:, :, 1, :]
                        else:
                            r0, r1 = rhs2[0][:], rhs2[1][:]
                        nc.tensor.matmul(
                            ps[:], wC_sb[wname][0][:, h * P:(h + 1) * P],
                            r0, start=True, stop=False,
                        )
                        nc.tensor.matmul(
                            ps[:], wC_sb[wname][1][:, h * P:(h + 1) * P],
                            r1, start=False, stop=True,
                        )
                        t = pC.tile([P, W], bf16, tag=f"{outtag}{h}",
                                    name=f"{outtag}{h}_{grp}")
                        nc.scalar.activation(
                            t[:], ps[:], Relu, bias=b_sb[bname][:, h:h + 1]
                        )
                        outs.append(t)
                    return outs

                xijT = mlp_layer(prodT, "xij_w", "xij_b", "xijT", True)
                u1T = mlp_layer(xcnT, "xcn_w1", "xcn_b1", "u1T", True)
                u2T = mlp_layer(u1T, "xcn_w2", "xcn_b2", "u2T", False)
                zT = []
                for h in range(2):
                    zb = pC.tile([P, W], bf16, tag=f"zb{h}", name=f"zb{h}_{grp}")
                    nc.vector.tensor_tensor(
                        out=zb[:], in0=u2T[h][:],
                        in1=beta_sb[:, 0:1].to_broadcast([P, W]), op=MUL,
                    )
                    zt = pC.tile([P, W], bf16, tag=f"zT{h}", name=f"zT{h}_{grp}")
                    nc.vector.tensor_tensor(
                        out=zt[:], in0=zb[:], in1=xijT[h][:], op=ADD
                    )
                    zT.append(zt)
                vT = mlp_layer(zT, "lin_w1", "lin_b1", "vT", False)
                pso = psO.tile([1, W], f32, tag="pso", name=f"pso{grp}")
                nc.tensor.matmul(
                    pso[:], lw2_sb[0][:], vT[0][:], start=True, stop=False
                )
                nc.tensor.matmul(
                    pso[:], lw2_sb[1][:], vT[1][:], start=False, stop=True
                )
                nc.scalar.activation(
                    out_row[0:1, grp * W:(grp + 1) * W], pso[:],
                    Ident, bias=lb2_sb[0:1, 0:1],
                )

            # ---- software-pipelined emission ----
            with tc.tile_pool(name="psB", bufs=1, space="PSUM") as psB:
                ps_map = {}

                def open_half(bh):
                    for b in range(bh * CGRP, (bh + 1) * CGRP):
                        ps_map[b] = psB.tile(
                            [P, C], f32, tag=f"psxcn{b % CGRP}",
                            name=f"psxcn{b}")

                def b_loads(bh, s):
                    for b in range(bh * CGRP, (bh + 1) * CGRP):
                        stage_b_load(b, s)

                def b_mms(bh, s):
                    for b in range(bh * CGRP, (bh + 1) * CGRP):
                        stage_b_mms(b, s, ps_map[b])

                with tc.tile_pool(name="stA", bufs=3) as pA, \
                     tc.tile_pool(name="psA", bufs=4, space="PSUM") as psA:
                    open_half(0)
                    b_loads(0, 0)
                    for g in range(8):
                        stage_a_group(g, pA, psA)
                    b_mms(0, 0)
                    stage_c_prod(0)
                    b_loads(0, 1)
                    for g in range(8, 16):
                        stage_a_group(g, pA, psA)
                with tc.tile_pool(name="psC", bufs=2, space="PSUM") as psC, \
                     tc.tile_pool(name="psO", bufs=1, space="PSUM") as psO:
                    b_mms(0, 1)
                    stage_c_prod(1)
                    for b in range(CGRP):
                        stage_b_finish(b, ps_map[b])
                    open_half(1)
                    b_loads(1, 0)
                    b_mms(1, 0)
                    b_loads(1, 1)
                    stage_c(0, psC, psO)
                    b_mms(1, 1)
                    for b in range(CGRP, 2 * CGRP):
                        stage_b_finish(b, ps_map[b])
                    stage_c(1, psC, psO)

            nc.sync.dma_start(out=out_d[:, :], in_=out_row[0:1, :])
            if DEBUG_DUMPS:
                nc.sync.dma_start(out=dbg["h_all"][:, :], in_=h_all[:])

    return _split_multi_waits(nc) if split_waits else nc


def kernel(**inputs):
    from concourse.bass_utils import run_bass_kernel_spmd

    if "nc" not in _CACHE:
        _CACHE["nc"] = _build()
    nc = _CACHE["nc"]

    x = np.ascontiguousarray(inputs["x"], dtype=np.float32)
    adj8 = np.ascontiguousarray(inputs["adj"]).astype(ml_dtypes.float8_e4m3)
    tar = np.asarray(inputs["tar_ei"]).astype(np.int32)

    def btile(b):
        return np.ascontiguousarray(np.asarray(b, dtype=np.float32).reshape(2, P).T)

    common = {
        "x": x.astype(ml_dtypes.bfloat16),
        "xT": np.ascontiguousarray(x.T).astype(ml_dtypes.bfloat16),
        "adj": adj8,
        "beta_bc": np.full((P, 1), np.asarray(inputs["beta"]).reshape(-1)[0],
                           dtype=np.float32),
        "lin_w2": np.ascontiguousarray(inputs["lin_w2"]).astype(ml_dtypes.bfloat16),
        "lin_b2": np.asarray(inputs["lin_b2"], dtype=np.float32).reshape(1, 1),
    }
    for n in ("xlin_w1", "xlin_w2"):
        common[n] = np.ascontiguousarray(inputs[n]).astype(ml_dtypes.bfloat16)
    for n in ("xcn_w1", "xcn_w2", "xij_w", "lin_w1"):
        common[n] = np.ascontiguousarray(inputs[n]).astype(ml_dtypes.bfloat16)
    common["ball"] = np.ascontiguousarray(np.concatenate(
        [btile(inputs[n]) for n in
         ("xlin_b1", "xlin_b2", "xcn_b1", "xcn_b2", "xij_b", "lin_b1")],
        axis=1))

    in_maps = []
    for c in range(NCORES):
        m = dict(common)
        m["idx"] = np.ascontiguousarray(tar[:, c * EL:(c + 1) * EL])
        in_maps.append(m)

    res = run_bass_kernel_spmd(
        nc, in_maps, core_ids=list(range(NCORES)), trace=TRACE
    )
    global LAST_RESULT
    LAST_RESULT = res
    out = np.concatenate(
        [res.results[c]["out"].reshape(EL, 1) for c in range(NCORES)], axis=0
    )
    return out.astype(np.float32)



# revision 2
# speedup vs baseline: 1.4534x; 1.4534x over previous
"""CNLinkPredictor Trainium2 kernel.

Edge-sharded across 8 NeuronCores (1024 target edges each); x, adj, and the
MLP weights are replicated. Per core:
  A) h = x + MLP(x) in transposed layout: host supplies xT, stage A is
     matmul-only on PE (bf16, 512-node moving dim), fused bias+ReLU on the
     scalar engine, residual on DVE into wide [128, 2048] hT tiles, then one
     xbar DMA-transpose per (c-half, 4-group) writes h back to natural
     layout (column order (hh, kt, c2): h[node=kt*128+p, ch=hh*128+c2]).
  B) the adjacency is BIT-PACKED on the host into u16 words with k-stride
     256: adjp[n, s*256+w] bit j = adj[n, s*4096 + j*256 + w].  Per edge
     block (128 edges): one indirect-DMA gather per endpoint fetches the
     full 1024-byte packed row pair; a DVE bitwise-AND gives the packed
     common-neighbor mask; ONE 2-byte xbar transpose per block moves it
     k-major (64 KB instead of the 1 MB an unpacked transpose would be).
     Per bit j: one DVE (shr j, and 1) op unpacks all 8 blocks at once
     (strided out AP groups columns into contiguous kt tiles), one DVE
     copy converts u16 0/1 -> bf16, then 16 matmuls accumulate
     xcnT[c, e] += h[kt]^T cn[kt] with h tiles stationary.  xcnT lands
     already transposed for stage C - no further transposes needed.
  C) xi*xj gathered per block (bf16 rows), DVE product, one packed xbar
     transpose per block; edge MLPs in transposed layout (512-edge
     groups), final [1, 1024] output row.

Hardware pitfalls this kernel works around:
  - This walrus build accepts at most ONE sync-wait per instruction
    (_apply_tile_patch splits the Tile tail drain; _split_multi_waits
    hoists extra waits onto same-engine NoOps).
  - Concurrent 4-byte DMA traffic corrupts in-flight 2-byte xbar
    DMA-transposes, so every steady-state transfer is <= 2 bytes/element
    (u16 packed adjacency, bf16 everything else); the few f32/int32 loads
    happen up front and the single f32 store happens after the last
    transpose.
  - The bitwise DVE path cannot cast dtypes, so the unpack is (shr, and)
    u16->u16 followed by a numeric-converting tensor_copy u16->bf16.
  - Indirect DMA consumes ONE index per out partition (the partition's
    whole span streams contiguously from the indexed row), so gathers are
    per-128-edge-block.
"""

import numpy as np
import ml_dtypes

N = 8192
C = 256
E = 8192
NCORES = 8
EL = E // NCORES          # edges per core
P = 128
NB = EL // P              # edge blocks per core (8)
NS = 2                    # k halves (4096 each)
NW = 256                  # u16 words per half per row
NJ = 16                   # bits per word
NKT = N // P              # 64 k tiles
AGRP = 512                # stage-A node group
NGW = 4                   # stage-A transpose super-groups (4 groups each)
CGRP = 4                  # stage-C blocks per group (512 edges)

_CACHE = {}
TRACE = False
LAST_RESULT = None


def _apply_tile_patch():
    """Split the Tile tail-drain's multi-sem wait onto individual SP nops."""
    from concourse.tile import TileContext
    from concourse.vector_clock import ScopedClock

    if getattr(TileContext, "_drain_patched", False):
        return

    def _patched(self, tick_clock, wait_clock):
        nc = self.nc
        collector = nc.sync.nop()
        wait_clock.add_sem_waits(
            collector.ins, ScopedClock({None: tick_clock.global_clock})
        )
        si = collector.ins.sync_info
        waits = list(si.on_wait) if si is not None and si.on_wait else []
        if si is not None and len(waits) > 1:
            name_to_handle = {h.name: h for h in self.sems.allocated().values()}
            si.on_wait = [waits[0]]
            for w in waits[1:]:
                op = {
                    "sem-ge-imm": "sem-ge",
                    "sem-eq-imm": "sem-eq",
                    "sem-le-imm": "sem-le",
                }.get(str(w.wait_mode), "sem-ge")
                nc.sync.nop().wait_op(name_to_handle[w.ant_name], w.wait_value, op)
        nc.sync.drain()
        nc.all_engine_barrier()
        assert self.sems is not None
        popped = nc._tile_sem_poison_stack.pop()
        assert popped is self._sem_poison
        nc.clear_and_free_semaphores(list(self.sems.allocated().values()))
        nc.all_engine_barrier()

    TileContext._drain_and_barrier = _patched
    TileContext._drain_patched = True


def _split_multi_waits(nc):
    """Hoist extra sync-waits onto same-engine NoOps (sequential waits ==
    ANDed waits); this walrus build allows one wait per instruction."""
    import concourse.mybir as mybir

    cnt = 0
    for fn in nc.m.functions:
        for bb in fn.blocks:
            out = []
            for inst in bb.instructions:
                si = getattr(inst, "sync_info", None)
                waits = list(si.on_wait) if si is not None and si.on_wait else []
                if len(waits) > 1:
                    for w in waits[:-1]:
                        nop = mybir.InstNoOp(name=f"ws-{cnt}", ins=[], outs=[])
                        cnt += 1
                        nop.engine = inst.engine
                        nop.sync_info = mybir.SyncInfo(on_wait=[w], on_update=[])
                        out.append(nop)
                    si.on_wait = [waits[-1]]
                out.append(inst)
            bb.instructions = out
    return nc


def _build(split_waits=True):
    import concourse.bass as bass
    import concourse.mybir as mybir
    from concourse.tile import TileContext

    _apply_tile_patch()

    f32 = mybir.dt.float32
    bf16 = mybir.dt.bfloat16
    u16 = mybir.dt.uint16
    i32 = mybir.dt.int32
    Relu = mybir.ActivationFunctionType.Relu
    Ident = mybir.ActivationFunctionType.Identity
    MUL = mybir.AluOpType.mult
    ADD = mybir.AluOpType.add
    ANDB = mybir.AluOpType.bitwise_and
    SHR = mybir.AluOpType.logical_shift_right

    nc = bass.Bass(num_swdge_queues=4)

    xT_d = nc.dram_tensor("xT", [C, N], bf16, kind="ExternalInput")
    x_d = nc.dram_tensor("x", [N, C], bf16, kind="ExternalInput")
    adjp_d = nc.dram_tensor("adjp", [N, NS * NW], u16, kind="ExternalInput")
    idx_d = nc.dram_tensor("idx", [2, EL], i32, kind="ExternalInput")
    # all bf16 matmul weights in one load: 6 [C,C] ws + lin_w2 padded to 2 cols
    WN = ["xlin_w1", "xlin_w2", "xcn_w1", "xcn_w2", "xij_w", "lin_w1"]
    wall_d = nc.dram_tensor("wall", [P, 12 * C + 2], bf16, kind="ExternalInput")
    # f32 consts: 6 bias pairs + beta + lin_b2
    bnames = ["xlin_b1", "xlin_b2", "xcn_b1", "xcn_b2", "xij_b", "lin_b1"]
    ball_d = nc.dram_tensor("ball", [P, 2 * len(bnames) + 2], f32,
                            kind="ExternalInput")
    out_d = nc.dram_tensor("out", [1, EL], f32, kind="ExternalOutput")

    _swq = [0]

    def _rr(inst):
        q = _swq[0] % 4
        _swq[0] += 1
        if q:
            inst.ins.queue = f"qPoolDynamic{q}"
        return inst

    with TileContext(nc) as tc:
        with (
            tc.tile_pool(name="const", bufs=1) as pK,
            tc.tile_pool(name="hpool", bufs=1) as pH,
            tc.tile_pool(name="adj", bufs=4) as pAdj,
            tc.tile_pool(name="cnp", bufs=4) as pCn,
            tc.tile_pool(name="unp", bufs=2) as pU,
            tc.tile_pool(name="cnT", bufs=3) as pT,
            tc.tile_pool(name="edge", bufs=1) as pE,
            tc.tile_pool(name="stC", bufs=2) as pC,
        ):
            # ---- constants ----
            idx_sb = pK.tile([P, 2 * NB], i32, tag="idx_sb", name="idx_sb")
            nc.sync.dma_start(
                out=idx_sb[:].rearrange("p (t b) -> p t b", t=2),
                in_=idx_d[:, :].rearrange("t (b p) -> p t b", p=P),
            )
            ii = [idx_sb[:, b:b + 1] for b in range(NB)]
            jj = [idx_sb[:, NB + b:NB + b + 1] for b in range(NB)]

            wall = pK.tile([P, 12 * C + 2], bf16, tag="wall", name="wall")
            nc.sync.dma_start(out=wall[:], in_=wall_d[:, :])
            w_sb = {}
            for q, n in enumerate(WN):
                w_sb[n] = [wall[:, (2 * q) * C:(2 * q + 1) * C],
                           wall[:, (2 * q + 1) * C:(2 * q + 2) * C]]
            lw2_sb = [wall[:, 12 * C:12 * C + 1], wall[:, 12 * C + 1:12 * C + 2]]

            ball = pK.tile([P, 2 * len(bnames) + 2], f32, tag="ball", name="ball")
            nc.sync.dma_start(out=ball[:], in_=ball_d[:, :])
            b_sb = {n: ball[:, 2 * q:2 * q + 2] for q, n in enumerate(bnames)}
            beta_sb = ball[:, 12:13]
            lb2_sb = ball[:, 13:14]

            out_row = pK.tile([1, EL], f32, tag="out_row", name="out_row")
            # natural-layout h: column = hh*N + kt*128 + c2 holds
            # h[node = kt*128 + p, channel = hh*128 + c2]
            h_all = pH.tile([P, 2 * N], bf16, tag="h_all", name="h_all")
            h_view = h_all[:].rearrange("p (hh kt c) -> p hh kt c", hh=2, c=P)

            # ---- stage-A xT loads (first half; rest after cnp transposes) ----
            xTw = {}
            for gw in range(2):
                for h in range(2):
                    t = pK.tile([P, 4 * AGRP], bf16, tag=f"xTw{gw}_{h}",
                                name=f"xTw{gw}_{h}")
                    nc.sync.dma_start(
                        out=t[:],
                        in_=xT_d[h * P:(h + 1) * P,
                                 gw * 4 * AGRP:(gw + 1) * 4 * AGRP],
                    )
                    xTw[(gw, h)] = t

            # ---- stage B: packed adjacency gathers + AND + transposes ----
            # T_all[p, b, 2s+wc, e] = word (s, wc*128+p) of edge (b,e)'s cn mask
            T_all = pH.tile([P, NB * 4 * P], u16, tag="T_all", name="T_all")
            T_view = T_all[:].rearrange("p (b ch e) -> p b ch e", b=NB, e=P)
            for b in range(NB):
                ai = pAdj.tile([P, NS * NW], u16, tag="ai", name=f"ai{b}")
                _rr(nc.gpsimd.indirect_dma_start(
                    out=ai[:], out_offset=None, in_=adjp_d[:, :],
                    in_offset=bass.IndirectOffsetOnAxis(ap=ii[b][:, :1], axis=0),
                ))
                aj = pAdj.tile([P, NS * NW], u16, tag="aj", name=f"aj{b}")
                _rr(nc.gpsimd.indirect_dma_start(
                    out=aj[:], out_offset=None, in_=adjp_d[:, :],
                    in_offset=bass.IndirectOffsetOnAxis(ap=jj[b][:, :1], axis=0),
                ))
                cnp = pCn.tile([P, NS * NW], u16, tag="cnp", name=f"cnp{b}")
                nc.vector.tensor_tensor(out=cnp[:], in0=ai[:], in1=aj[:], op=ANDB)
                nc.sync.dma_start_transpose(out=T_view[:, b, :, :], in_=cnp[:])

            # ---- remaining xT loads ----
            for gw in range(2, NGW):
                for h in range(2):
                    t = pK.tile([P, 4 * AGRP], bf16, tag=f"xTw{gw}_{h}",
                                name=f"xTw{gw}_{h}")
                    nc.sync.dma_start(
                        out=t[:],
                        in_=xT_d[h * P:(h + 1) * P,
                                 gw * 4 * AGRP:(gw + 1) * 4 * AGRP],
                    )
                    xTw[(gw, h)] = t

            # ---- stage A ----
            with tc.tile_pool(name="stA", bufs=3) as pA, \
                 tc.tile_pool(name="hT", bufs=2) as pHT, \
                 tc.tile_pool(name="psA", bufs=4, space="PSUM") as psA:
                hTw = {}
                for g in range(4 * NGW):
                    gw, gl = g // 4, g % 4
                    if gl == 0:
                        for h in range(2):
                            hTw[h] = pHT.tile([P, 4 * AGRP], bf16,
                                              tag=f"hTw{h}", name=f"hTw{gw}_{h}")
                    xT = [xTw[(gw, h)][:, gl * AGRP:(gl + 1) * AGRP]
                          for h in range(2)]
                    y1T = []
                    for h in range(2):
                        ps = psA.tile([P, AGRP], f32, tag="psmm",
                                      name=f"psA1_{g}{h}")
                        nc.tensor.matmul(
                            ps[:], w_sb["xlin_w1"][0][:, h * P:(h + 1) * P],
                            xT[0], start=True, stop=False,
                        )
                        nc.tensor.matmul(
                            ps[:], w_sb["xlin_w1"][1][:, h * P:(h + 1) * P],
                            xT[1], start=False, stop=True,
                        )
                        t = pA.tile([P, AGRP], bf16, tag=f"y1T{h}",
                                    name=f"y1T{h}_{g}")
                        nc.scalar.activation(
                            t[:], ps[:], Relu, bias=b_sb["xlin_b1"][:, h:h + 1]
                        )
                        y1T.append(t)
                    for h in range(2):
                        ps = psA.tile([P, AGRP], f32, tag="psmm",
                                      name=f"psA2_{g}{h}")
                        nc.tensor.matmul(
                            ps[:], w_sb["xlin_w2"][0][:, h * P:(h + 1) * P],
                            y1T[0][:], start=True, stop=False,
                        )
                        nc.tensor.matmul(
                            ps[:], w_sb["xlin_w2"][1][:, h * P:(h + 1) * P],
                            y1T[1][:], start=False, stop=True,
                        )
                        y2 = pA.tile([P, AGRP], bf16, tag="y2T", name=f"y2T{h}_{g}")
                        nc.scalar.activation(
                            y2[:], ps[:], Relu, bias=b_sb["xlin_b2"][:, h:h + 1]
                        )
                        nc.vector.tensor_tensor(
                            out=hTw[h][:, gl * AGRP:(gl + 1) * AGRP],
                            in0=xT[h], in1=y2[:], op=ADD,
                        )
                    if gl == 3:
                        for h in range(2):
                            nc.sync.dma_start_transpose(
                                out=h_view[:, h, gw * 16:(gw + 1) * 16, :],
                                in_=hTw[h][:],
                            )

            # ---- stage C gathers + products (overlap stage A tail) ----
            prodT_all = pH.tile([P, NB * 2 * P], bf16, tag="prodT",
                                name="prodT_all")
            prodT_v = prodT_all[:].rearrange("p (b ch e) -> p b ch e",
                                             b=NB, e=P)
            prods = []
            for b in range(NB):
                xi = pE.tile([P, C], bf16, tag=f"xi{b}", name=f"xi{b}")
                _rr(nc.gpsimd.indirect_dma_start(
                    out=xi[:], out_offset=None, in_=x_d[:, :],
                    in_offset=bass.IndirectOffsetOnAxis(ap=ii[b][:, :1], axis=0),
                ))
                xj = pE.tile([P, C], bf16, tag=f"xj{b}", name=f"xj{b}")
                _rr(nc.gpsimd.indirect_dma_start(
                    out=xj[:], out_offset=None, in_=x_d[:, :],
                    in_offset=bass.IndirectOffsetOnAxis(ap=jj[b][:, :1], axis=0),
                ))
                pt = pE.tile([P, C], bf16, tag=f"prod{b}", name=f"prod{b}")
                prods.append((pt, xi, xj))

            # ---- stage B: unpack + matmuls (h stationary, cn moving) ----
            with tc.tile_pool(name="psB", bufs=1, space="PSUM") as psB:
                ps_x = {}
                for hh in range(2):
                    for eh in range(2):
                        ps_x[(hh, eh)] = psB.tile(
                            [P, EL // 2], f32, tag=f"psx{hh}{eh}",
                            name=f"psx{hh}{eh}")
                for j in range(NJ):
                    uj = pU.tile([P, 4 * EL // 2 * 2], u16, tag="uj",
                                 name=f"uj{j}")
                    # out col = s*2048 + wc*1024 + b*128 + e; in iterates
                    # (b, s, wc, e) to match T_all's (b, ch=(s,wc), e)
                    nc.vector.tensor_scalar(
                        out=uj[:].rearrange("p (s wc b e) -> p b s wc e",
                                            s=2, wc=2, e=P),
                        in0=T_view[:, :, :, :].rearrange(
                            "p b (s wc) e -> p b s wc e", s=2),
                        scalar1=j, scalar2=1, op0=SHR, op1=ANDB,
                    )
                    cj = pT.tile([P, 4 * EL // 2 * 2], bf16, tag="cj",
                                 name=f"cj{j}")
                    nc.vector.tensor_copy(cj[:], uj[:])
                    for s in range(2):
                        for wc in range(2):
                            kt = s * 32 + 2 * j + wc
                            rhs = cj[:, (2 * s + wc) * EL:(2 * s + wc + 1) * EL]
                            for hh in range(2):
                                for eh in range(2):
                                    nc.tensor.matmul(
                                        ps_x[(hh, eh)][:],
                                        h_view[:, hh, kt, :],
                                        rhs[:, eh * 512:(eh + 1) * 512],
                                        start=(j == 0 and s == 0 and wc == 0),
                                        stop=(j == NJ - 1 and s == 1
                                              and wc == 1),
                                    )
                    if j == 1:
                        # DVE slack early in the j-loop: edge products
                        for pt, xi, xj in prods:
                            nc.vector.tensor_tensor(
                                out=pt[:], in0=xi[:], in1=xj[:], op=MUL
                            )
                        for b in range(NB):
                            nc.sync.dma_start_transpose(
                                out=prodT_v[:, b, :, :], in_=prods[b][0][:],
                            )
                xcn_sb = pH.tile([P, 2 * EL], bf16, tag="xcn", name="xcn_sb")
                for hh in range(2):
                    for eh in range(2):
                        nc.vector.tensor_copy(
                            xcn_sb[:, hh * EL + eh * 512:
                                   hh * EL + (eh + 1) * 512],
                            ps_x[(hh, eh)][:])

            # ---- stage C ----
            with tc.tile_pool(name="psC", bufs=2, space="PSUM") as psC, \
                 tc.tile_pool(name="psO", bufs=1, space="PSUM") as psO:
                for grp in range(EL // 512):
                    W = 512

                    def mlp_layer(rhs_pair, wname, bname, outtag):
                        outs = []
                        for h in range(2):
                            ps = psC.tile([P, W], f32, tag="psc",
                                          name=f"psc_{grp}_{outtag}{h}")
                            nc.tensor.matmul(
                                ps[:], w_sb[wname][0][:, h * P:(h + 1) * P],
                                rhs_pair[0], start=True, stop=False,
                            )
                            nc.tensor.matmul(
                                ps[:], w_sb[wname][1][:, h * P:(h + 1) * P],
                                rhs_pair[1], start=False, stop=True,
                            )
                            t = pC.tile([P, W], bf16, tag=f"{outtag}{h}",
                                        name=f"{outtag}{h}_{grp}")
                            nc.scalar.activation(
                                t[:], ps[:], Relu, bias=b_sb[bname][:, h:h + 1]
                            )
                            outs.append(t)
                        return outs

                    prod_rhs = [
                        prodT_v[:, grp * CGRP:(grp + 1) * CGRP, ch, :]
                        for ch in range(2)
                    ]
                    xcn_rhs = [
                        xcn_sb[:, ch * EL + grp * W:ch * EL + (grp + 1) * W]
                        for ch in range(2)
                    ]
                    xijT = mlp_layer(prod_rhs, "xij_w", "xij_b", "xijT")
                    u1T = mlp_layer(xcn_rhs, "xcn_w1", "xcn_b1", "u1T")
                    u2T = mlp_layer([u1T[0][:], u1T[1][:]],
                                    "xcn_w2", "xcn_b2", "u2T")
                    zT = []
                    for h in range(2):
                        zb = pC.tile([P, W], bf16, tag=f"zb{h}",
                                     name=f"zb{h}_{grp}")
                        nc.vector.tensor_tensor(
                            out=zb[:], in0=u2T[h][:],
                            in1=beta_sb.to_broadcast([P, W]), op=MUL,
                        )
                        zt = pC.tile([P, W], bf16, tag=f"zT{h}",
                                     name=f"zT{h}_{grp}")
                        nc.vector.tensor_tensor(
                            out=zt[:], in0=zb[:], in1=xijT[h][:], op=ADD
                        )
                        zT.append(zt)
                    vT = mlp_layer([zT[0][:], zT[1][:]],
                                   "lin_w1", "lin_b1", "vT")
                    pso = psO.tile([1, W], f32, tag="pso", name=f"pso{grp}")
                    nc.tensor.matmul(
                        pso[:], lw2_sb[0][:], vT[0][:], start=True, stop=False
                    )
                    nc.tensor.matmul(
                        pso[:], lw2_sb[1][:], vT[1][:], start=False, stop=True
                    )
                    nc.scalar.activation(
                        out_row[0:1, grp * W:(grp + 1) * W], pso[:],
                        Ident, bias=lb2_sb[0:1, 0:1],
                    )

            nc.sync.dma_start(out=out_d[:, :], in_=out_row[0:1, :])

    return _split_multi_waits(nc) if split_waits else nc


def kernel(**inputs):
    from concourse.bass_utils import run_bass_kernel_spmd

    if "nc" not in _CACHE:
        _CACHE["nc"] = _build()
    nc = _CACHE["nc"]

    x = np.ascontiguousarray(inputs["x"], dtype=np.float32)
    adj = np.asarray(inputs["adj"])
    # pack adjacency bits into u16 words, k-stride 256 within each 4096-half:
    # adjp[n, s*256+w] bit j = adj[n, s*4096 + j*256 + w]
    A = (adj != 0).astype(np.uint16).reshape(N, 2, 16, 256)
    adjp = np.zeros((N, 2, 256), np.uint16)
    for j in range(16):
        adjp |= A[:, :, j, :] << j
    adjp = np.ascontiguousarray(adjp.reshape(N, 512))
    tar = np.asarray(inputs["tar_ei"]).astype(np.int32)

    def btile(b):
        return np.ascontiguousarray(
            np.asarray(b, dtype=np.float32).reshape(2, P).T)

    WN = ["xlin_w1", "xlin_w2", "xcn_w1", "xcn_w2", "xij_w", "lin_w1"]
    wall = np.zeros((P, 12 * C + 2), np.float32)
    for q, n in enumerate(WN):
        w = np.asarray(inputs[n], dtype=np.float32)  # [C, C]
        # column q*2C..: [k-half][p, n2] with k = kh*128 + p
        wall[:, (2 * q) * C:(2 * q + 1) * C] = w[:P, :]
        wall[:, (2 * q + 1) * C:(2 * q + 2) * C] = w[P:, :]
    lw2 = np.asarray(inputs["lin_w2"], dtype=np.float32).reshape(C, 1)
    wall[:, 12 * C] = lw2[:P, 0]
    wall[:, 12 * C + 1] = lw2[P:, 0]

    bnames = ["xlin_b1", "xlin_b2", "xcn_b1", "xcn_b2", "xij_b", "lin_b1"]
    ball = np.zeros((P, 2 * len(bnames) + 2), np.float32)
    for q, n in enumerate(bnames):
        ball[:, 2 * q:2 * q + 2] = btile(inputs[n])
    ball[:, 12] = np.asarray(inputs["beta"]).reshape(-1)[0]
    ball[:, 13] = np.asarray(inputs["lin_b2"]).reshape(-1)[0]

    common = {
        "x": x.astype(ml_dtypes.bfloat16),
        "xT": np.ascontiguousarray(x.T).astype(ml_dtypes.bfloat16),
        "adjp": adjp,
        "wall": np.ascontiguousarray(wall).astype(ml_dtypes.bfloat16),
        "ball": np.ascontiguousarray(ball),
    }

    in_maps = []
    for c in range(NCORES):
        m = dict(common)
        m["idx"] = np.ascontiguousarray(tar[:, c * EL:(c + 1) * EL])
        in_maps.append(m)

    res = run_bass_kernel_spmd(
        nc, in_maps, core_ids=list(range(NCORES)), trace=TRACE
    )
    global LAST_RESULT
    LAST_RESULT = res
    out = np.concatenate(
        [res.results[c]["out"].reshape(EL, 1) for c in range(NCORES)], axis=0
    )
    return out.astype(np.float32)


# revision 3
# speedup vs baseline: 1.5089x; 1.0382x over previous
"""CNLinkPredictor Trainium2 kernel.

Edge-sharded across 8 NeuronCores (1024 target edges each); x, adj, and the
MLP weights are replicated. Per core:
  A) h = x + MLP(x) in transposed layout: host supplies xT, stage A is
     matmul-only on PE (bf16, 512-node moving dim), fused bias+ReLU on the
     scalar engine, residual on DVE into wide [128, 2048] hT tiles, then one
     xbar DMA-transpose per (c-half, 4-group) writes h back to natural
     layout (column order (hh, kt, c2): h[node=kt*128+p, ch=hh*128+c2]).
  B) the adjacency is BIT-PACKED on the host into u16 words with k-stride
     256: adjp[n, s*256+w] bit j = adj[n, s*4096 + j*256 + w].  Per edge
     block (128 edges): one indirect-DMA gather per endpoint fetches the
     full 1024-byte packed row pair; a DVE bitwise-AND gives the packed
     common-neighbor mask; ONE 2-byte xbar transpose per block moves it
     k-major (64 KB instead of the 1 MB an unpacked transpose would be).
     Per (bit j, half s): one DVE (shr j, and 1) op unpacks all 8 blocks at
     once (strided out AP groups columns into contiguous kt tiles), one DVE
     copy converts u16 0/1 -> bf16, then 8 matmuls accumulate
     xcnT[c, e] += h[kt]^T cn[kt] with h tiles stationary.  xcnT lands
     already transposed for stage C - no further transposes needed.
  C) xi*xj gathered per block (bf16 rows), DVE product, one packed xbar
     transpose per block; edge MLPs in transposed layout (512-edge
     groups).  The xij layer (independent of xcn) is emitted INTO the
     stage-B j-loop so PE/Act digest it during B; only the xcn-dependent
     chain (u1 -> u2 -> z -> v -> out) trails the last B matmul.

Scheduling rules this kernel follows (learned from TimelineSim traces):
  - Engine SEQs are in-order FIFOs and a waiting instruction parks on the
    queue head: all dependency-free DMAs (xT loads) are emitted before any
    dependent transpose on the same queue.
  - PSUM is 8 banks: stage A uses 4 (scoped), then stage B accumulators 4
    + stage C layer tiles 3 + output 1 coexist.

Hardware pitfalls this kernel works around:
  - This walrus build accepts at most ONE sync-wait per instruction
    (_apply_tile_patch splits the Tile tail drain; _split_multi_waits
    hoists extra waits onto same-engine NoOps).
  - Concurrent 4-byte DMA traffic corrupts in-flight 2-byte xbar
    DMA-transposes, so every steady-state transfer is <= 2 bytes/element
    (u16 packed adjacency, bf16 everything else); the few f32/int32 loads
    happen up front and the single f32 store happens after the last
    transpose.
  - The bitwise DVE path cannot cast dtypes, so the unpack is (shr, and)
    u16->u16 followed by a numeric-converting tensor_copy u16->bf16.
  - Indirect DMA consumes ONE index per out partition (the partition's
    whole span streams contiguously from the indexed row), so gathers are
    per-128-edge-block.
"""

import numpy as np
import ml_dtypes

N = 8192
C = 256
E = 8192
NCORES = 8
EL = E // NCORES          # edges per core
P = 128
NB = EL // P              # edge blocks per core (8)
NS = 2                    # k halves (4096 each)
NW = 256                  # u16 words per half per row
NJ = 16                   # bits per word
AGRP = 512                # stage-A node group
NGW = 4                   # stage-A transpose super-groups (4 groups each)
CGRP = 4                  # stage-C blocks per group (512 edges)

_CACHE = {}
TRACE = False
LAST_RESULT = None


def _apply_tile_patch():
    """Split the Tile tail-drain's multi-sem wait onto individual SP nops."""
    from concourse.tile import TileContext
    from concourse.vector_clock import ScopedClock

    if getattr(TileContext, "_drain_patched", False):
        return

    def _patched(self, tick_clock, wait_clock):
        nc = self.nc
        collector = nc.sync.nop()
        wait_clock.add_sem_waits(
            collector.ins, ScopedClock({None: tick_clock.global_clock})
        )
        si = collector.ins.sync_info
        waits = list(si.on_wait) if si is not None and si.on_wait else []
        if si is not None and len(waits) > 1:
            name_to_handle = {h.name: h for h in self.sems.allocated().values()}
            si.on_wait = [waits[0]]
            for w in waits[1:]:
                op = {
                    "sem-ge-imm": "sem-ge",
                    "sem-eq-imm": "sem-eq",
                    "sem-le-imm": "sem-le",
                }.get(str(w.wait_mode), "sem-ge")
                nc.sync.nop().wait_op(name_to_handle[w.ant_name], w.wait_value, op)
        nc.sync.drain()
        nc.all_engine_barrier()
        assert self.sems is not None
        popped = nc._tile_sem_poison_stack.pop()
        assert popped is self._sem_poison
        nc.clear_and_free_semaphores(list(self.sems.allocated().values()))
        nc.all_engine_barrier()

    TileContext._drain_and_barrier = _patched
    TileContext._drain_patched = True


def _split_multi_waits(nc):
    """Hoist extra sync-waits onto same-engine NoOps (sequential waits ==
    ANDed waits); this walrus build allows one wait per instruction."""
    import concourse.mybir as mybir

    cnt = 0
    for fn in nc.m.functions:
        for bb in fn.blocks:
            out = []
            for inst in bb.instructions:
                si = getattr(inst, "sync_info", None)
                waits = list(si.on_wait) if si is not None and si.on_wait else []
                if len(waits) > 1:
                    for w in waits[:-1]:
                        nop = mybir.InstNoOp(name=f"ws-{cnt}", ins=[], outs=[])
                        cnt += 1
                        nop.engine = inst.engine
                        nop.sync_info = mybir.SyncInfo(on_wait=[w], on_update=[])
                        out.append(nop)
                    si.on_wait = [waits[-1]]
                out.append(inst)
            bb.instructions = out
    return nc


def _build(split_waits=True):
    import concourse.bass as bass
    import concourse.mybir as mybir
    from concourse.tile import TileContext

    _apply_tile_patch()

    f32 = mybir.dt.float32
    bf16 = mybir.dt.bfloat16
    u16 = mybir.dt.uint16
    i32 = mybir.dt.int32
    Relu = mybir.ActivationFunctionType.Relu
    Ident = mybir.ActivationFunctionType.Identity
    MUL = mybir.AluOpType.mult
    ADD = mybir.AluOpType.add
    ANDB = mybir.AluOpType.bitwise_and
    SHR = mybir.AluOpType.logical_shift_right

    nc = bass.Bass(num_swdge_queues=4)

    xT_d = nc.dram_tensor("xT", [C, N], bf16, kind="ExternalInput")
    x_d = nc.dram_tensor("x", [N, C], bf16, kind="ExternalInput")
    adjp_d = nc.dram_tensor("adjp", [N, NS * NW], u16, kind="ExternalInput")
    idx_d = nc.dram_tensor("idx", [2, EL], i32, kind="ExternalInput")
    # all bf16 matmul weights in one load: 6 [C,C] ws + lin_w2 padded to 2 cols
    WN = ["xlin_w1", "xlin_w2", "xcn_w1", "xcn_w2", "xij_w", "lin_w1"]
    wall_d = nc.dram_tensor("wall", [P, 12 * C + 2], bf16, kind="ExternalInput")
    # f32 consts: 6 bias pairs + beta + lin_b2
    bnames = ["xlin_b1", "xlin_b2", "xcn_b1", "xcn_b2", "xij_b", "lin_b1"]
    ball_d = nc.dram_tensor("ball", [P, 2 * len(bnames) + 2], f32,
                            kind="ExternalInput")
    out_d = nc.dram_tensor("out", [1, EL], f32, kind="ExternalOutput")

    _swq = [0]

    def _rr(inst):
        q = _swq[0] % 4
        _swq[0] += 1
        if q:
            inst.ins.queue = f"qPoolDynamic{q}"
        return inst

    with TileContext(nc) as tc:
        with (
            tc.tile_pool(name="const", bufs=1) as pK,
            tc.tile_pool(name="hpool", bufs=1) as pH,
            tc.tile_pool(name="adj", bufs=4) as pAdj,
            tc.tile_pool(name="cnp", bufs=4) as pCn,
            tc.tile_pool(name="unp", bufs=2) as pU,
            tc.tile_pool(name="cnT", bufs=3) as pT,
            tc.tile_pool(name="edge", bufs=1) as pE,
            tc.tile_pool(name="stC", bufs=2) as pC,
        ):
            # ---- constants ----
            idx_sb = pK.tile([P, 2 * NB], i32, tag="idx_sb", name="idx_sb")
            nc.sync.dma_start(
                out=idx_sb[:].rearrange("p (t b) -> p t b", t=2),
                in_=idx_d[:, :].rearrange("t (b p) -> p t b", p=P),
            )
            ii = [idx_sb[:, b:b + 1] for b in range(NB)]
            jj = [idx_sb[:, NB + b:NB + b + 1] for b in range(NB)]

            wall = pK.tile([P, 12 * C + 2], bf16, tag="wall", name="wall")
            nc.sync.dma_start(out=wall[:], in_=wall_d[:, :])
            w_sb = {}
            for q, n in enumerate(WN):
                w_sb[n] = [wall[:, (2 * q) * C:(2 * q + 1) * C],
                           wall[:, (2 * q + 1) * C:(2 * q + 2) * C]]
            lw2_sb = [wall[:, 12 * C:12 * C + 1], wall[:, 12 * C + 1:12 * C + 2]]

            ball = pK.tile([P, 2 * len(bnames) + 2], f32, tag="ball", name="ball")
            nc.sync.dma_start(out=ball[:], in_=ball_d[:, :])
            b_sb = {n: ball[:, 2 * q:2 * q + 2] for q, n in enumerate(bnames)}
            beta_sb = ball[:, 12:13]
            lb2_sb = ball[:, 13:14]

            out_row = pK.tile([1, EL], f32, tag="out_row", name="out_row")
            # natural-layout h: column = hh*N + kt*128 + c2 holds
            # h[node = kt*128 + p, channel = hh*128 + c2]
            h_all = pH.tile([P, 2 * N], bf16, tag="h_all", name="h_all")
            h_view = h_all[:].rearrange("p (hh kt c) -> p hh kt c", hh=2, c=P)

            # ---- ALL stage-A xT loads first: dependency-free on SP.SEQ ----
            xTw = {}
            for gw in range(NGW):
                for h in range(2):
                    t = pK.tile([P, 4 * AGRP], bf16, tag=f"xTw{gw}_{h}",
                                name=f"xTw{gw}_{h}")
                    nc.sync.dma_start(
                        out=t[:],
                        in_=xT_d[h * P:(h + 1) * P,
                                 gw * 4 * AGRP:(gw + 1) * 4 * AGRP],
                    )
                    xTw[(gw, h)] = t

            # ---- stage B gathers + AND + packed transposes ----
            # T_all[p, b, 2s+wc, e] = word (s, wc*128+p) of edge (b,e)'s cn mask
            T_all = pH.tile([P, NB * 4 * P], u16, tag="T_all", name="T_all")
            T_view = T_all[:].rearrange("p (b ch e) -> p b ch e", b=NB, e=P)
            for b in range(NB):
                ai = pAdj.tile([P, NS * NW], u16, tag="ai", name=f"ai{b}")
                _rr(nc.gpsimd.indirect_dma_start(
                    out=ai[:], out_offset=None, in_=adjp_d[:, :],
                    in_offset=bass.IndirectOffsetOnAxis(ap=ii[b][:, :1], axis=0),
                ))
                aj = pAdj.tile([P, NS * NW], u16, tag="aj", name=f"aj{b}")
                _rr(nc.gpsimd.indirect_dma_start(
                    out=aj[:], out_offset=None, in_=adjp_d[:, :],
                    in_offset=bass.IndirectOffsetOnAxis(ap=jj[b][:, :1], axis=0),
                ))
                cnp = pCn.tile([P, NS * NW], u16, tag="cnp", name=f"cnp{b}")
                nc.vector.tensor_tensor(out=cnp[:], in0=ai[:], in1=aj[:], op=ANDB)
                nc.sync.dma_start_transpose(out=T_view[:, b, :, :], in_=cnp[:])

            # ---- stage C gathers (Pool queue, after adjacency gathers) ----
            exi, exj = [], []
            for b in range(NB):
                xi = pE.tile([P, C], bf16, tag=f"xi{b}", name=f"xi{b}")
                _rr(nc.gpsimd.indirect_dma_start(
                    out=xi[:], out_offset=None, in_=x_d[:, :],
                    in_offset=bass.IndirectOffsetOnAxis(ap=ii[b][:, :1], axis=0),
                ))
                xj = pE.tile([P, C], bf16, tag=f"xj{b}", name=f"xj{b}")
                _rr(nc.gpsimd.indirect_dma_start(
                    out=xj[:], out_offset=None, in_=x_d[:, :],
                    in_offset=bass.IndirectOffsetOnAxis(ap=jj[b][:, :1], axis=0),
                ))
                exi.append(xi)
                exj.append(xj)

            # ---- stage A ----
            with tc.tile_pool(name="stA", bufs=3) as pA, \
                 tc.tile_pool(name="hT", bufs=2) as pHT, \
                 tc.tile_pool(name="psA", bufs=4, space="PSUM") as psA:
                hTw = {}
                for g in range(4 * NGW):
                    gw, gl = g // 4, g % 4
                    if gl == 0:
                        for h in range(2):
                            hTw[h] = pHT.tile([P, 4 * AGRP], bf16,
                                              tag=f"hTw{h}", name=f"hTw{gw}_{h}")
                    xT = [xTw[(gw, h)][:, gl * AGRP:(gl + 1) * AGRP]
                          for h in range(2)]
                    y1T = []
                    for h in range(2):
                        ps = psA.tile([P, AGRP], f32, tag="psmm",
                                      name=f"psA1_{g}{h}")
                        nc.tensor.matmul(
                            ps[:], w_sb["xlin_w1"][0][:, h * P:(h + 1) * P],
                            xT[0], start=True, stop=False,
                        )
                        nc.tensor.matmul(
                            ps[:], w_sb["xlin_w1"][1][:, h * P:(h + 1) * P],
                            xT[1], start=False, stop=True,
                        )
                        t = pA.tile([P, AGRP], bf16, tag=f"y1T{h}",
                                    name=f"y1T{h}_{g}")
                        nc.scalar.activation(
                            t[:], ps[:], Relu, bias=b_sb["xlin_b1"][:, h:h + 1]
                        )
                        y1T.append(t)
                    for h in range(2):
                        ps = psA.tile([P, AGRP], f32, tag="psmm",
                                      name=f"psA2_{g}{h}")
                        nc.tensor.matmul(
                            ps[:], w_sb["xlin_w2"][0][:, h * P:(h + 1) * P],
                            y1T[0][:], start=True, stop=False,
                        )
                        nc.tensor.matmul(
                            ps[:], w_sb["xlin_w2"][1][:, h * P:(h + 1) * P],
                            y1T[1][:], start=False, stop=True,
                        )
                        y2 = pA.tile([P, AGRP], bf16, tag="y2T", name=f"y2T{h}_{g}")
                        nc.scalar.activation(
                            y2[:], ps[:], Relu, bias=b_sb["xlin_b2"][:, h:h + 1]
                        )
                        nc.vector.tensor_tensor(
                            out=hTw[h][:, gl * AGRP:(gl + 1) * AGRP],
                            in0=xT[h], in1=y2[:], op=ADD,
                        )
                    if gl == 3:
                        for h in range(2):
                            nc.sync.dma_start_transpose(
                                out=h_view[:, h, gw * 16:(gw + 1) * 16, :],
                                in_=hTw[h][:],
                            )

            # ---- edge products + transposes (DVE/SP, ready mid j-loop) ----
            prodT_all = pH.tile([P, NB * 2 * P], bf16, tag="prodT",
                                name="prodT_all")
            prodT_v = prodT_all[:].rearrange("p (b ch e) -> p b ch e",
                                             b=NB, e=P)
            for b in range(NB):
                pt = pE.tile([P, C], bf16, tag=f"prod{b}", name=f"prod{b}")
                nc.vector.tensor_tensor(
                    out=pt[:], in0=exi[b][:], in1=exj[b][:], op=MUL
                )
                nc.sync.dma_start_transpose(out=prodT_v[:, b, :, :], in_=pt[:])

            # ---- stage B j-loop + interleaved stage-C xij layer ----
            with tc.tile_pool(name="psB", bufs=1, space="PSUM") as psB, \
                 tc.tile_pool(name="psC", bufs=3, space="PSUM") as psC, \
                 tc.tile_pool(name="psO", bufs=1, space="PSUM") as psO:
                ps_x = {}
                for hh in range(2):
                    for eh in range(2):
                        ps_x[(hh, eh)] = psB.tile(
                            [P, EL // 2], f32, tag=f"psx{hh}{eh}",
                            name=f"psx{hh}{eh}")

                def mlp_layer(grp, rhs_pair, wname, bname, outtag):
                    W = 512
                    outs = []
                    for h in range(2):
                        ps = psC.tile([P, W], f32, tag="psc",
                                      name=f"psc_{grp}_{outtag}{h}")
                        nc.tensor.matmul(
                            ps[:], w_sb[wname][0][:, h * P:(h + 1) * P],
                            rhs_pair[0], start=True, stop=False,
                        )
                        nc.tensor.matmul(
                            ps[:], w_sb[wname][1][:, h * P:(h + 1) * P],
                            rhs_pair[1], start=False, stop=True,
                        )
                        t = pC.tile([P, W], bf16, tag=f"{outtag}{h}",
                                    name=f"{outtag}{h}_{grp}")
                        nc.scalar.activation(
                            t[:], ps[:], Relu, bias=b_sb[bname][:, h:h + 1]
                        )
                        outs.append(t)
                    return outs

                xijT = {}
                for j in range(NJ):
                    for s in range(2):
                        uj = pU.tile([P, 2 * EL], u16, tag="uj",
                                     name=f"uj{j}_{s}")
                        # out col = wc*1024 + b*128 + e; in iterates
                        # (b, wc, e) matching T_all's (b, ch=2s+wc, e)
                        nc.vector.tensor_scalar(
                            out=uj[:].rearrange("p (wc b e) -> p b wc e",
                                                wc=2, e=P),
                            in0=T_view[:, :, 2 * s:2 * s + 2, :],
                            scalar1=j, scalar2=1, op0=SHR, op1=ANDB,
                        )
                        cj = pT.tile([P, 2 * EL], bf16, tag="cj",
                                     name=f"cj{j}_{s}")
                        nc.vector.tensor_copy(cj[:], uj[:])
                        for wc in range(2):
                            kt = s * 32 + 2 * j + wc
                            for hh in range(2):
                                for eh in range(2):
                                    nc.tensor.matmul(
                                        ps_x[(hh, eh)][:],
                                        h_view[:, hh, kt, :],
                                        cj[:, wc * EL + eh * 512:
                                           wc * EL + (eh + 1) * 512],
                                        start=(j == 0 and s == 0 and wc == 0),
                                        stop=(j == NJ - 1 and s == 1
                                              and wc == 1),
                                    )
                    if j == 9:
                        # PE/Act slack inside the j-loop: xij layer for both
                        # 512-edge groups (depends only on prodT)
                        for grp in range(EL // 512):
                            xijT[grp] = mlp_layer(
                                grp,
                                [prodT_v[:, grp * CGRP:(grp + 1) * CGRP, ch, :]
                                 for ch in range(2)],
                                "xij_w", "xij_b", "xijT")

                xcn_sb = pH.tile([P, 2 * EL], bf16, tag="xcn", name="xcn_sb")
                for hh in range(2):
                    for eh in range(2):
                        nc.vector.tensor_copy(
                            xcn_sb[:, hh * EL + eh * 512:
                                   hh * EL + (eh + 1) * 512],
                            ps_x[(hh, eh)][:])

                # ---- stage C xcn-dependent chain ----
                for grp in range(EL // 512):
                    W = 512
                    xcn_rhs = [
                        xcn_sb[:, ch * EL + grp * W:ch * EL + (grp + 1) * W]
                        for ch in range(2)
                    ]
                    u1T = mlp_layer(grp, xcn_rhs, "xcn_w1", "xcn_b1", "u1T")
                    u2T = mlp_layer(grp, [u1T[0][:], u1T[1][:]],
                                    "xcn_w2", "xcn_b2", "u2T")
                    zT = []
                    for h in range(2):
                        zb = pC.tile([P, W], bf16, tag=f"zb{h}",
                                     name=f"zb{h}_{grp}")
                        nc.vector.tensor_tensor(
                            out=zb[:], in0=u2T[h][:],
                            in1=beta_sb.to_broadcast([P, W]), op=MUL,
                        )
                        zt = pC.tile([P, W], bf16, tag=f"zT{h}",
                                     name=f"zT{h}_{grp}")
                        nc.vector.tensor_tensor(
                            out=zt[:], in0=zb[:], in1=xijT[grp][h][:], op=ADD
                        )
                        zT.append(zt)
                    vT = mlp_layer(grp, [zT[0][:], zT[1][:]],
                                   "lin_w1", "lin_b1", "vT")
                    pso = psO.tile([1, W], f32, tag="pso", name=f"pso{grp}")
                    nc.tensor.matmul(
                        pso[:], lw2_sb[0][:], vT[0][:], start=True, stop=False
                    )
                    nc.tensor.matmul(
                        pso[:], lw2_sb[1][:], vT[1][:], start=False, stop=True
                    )
                    nc.scalar.activation(
                        out_row[0:1, grp * W:(grp + 1) * W], pso[:],
                        Ident, bias=lb2_sb[0:1, 0:1],
                    )

            nc.sync.dma_start(out=out_d[:, :], in_=out_row[0:1, :])

    return _split_multi_waits(nc) if split_waits else nc


def kernel(**inputs):
    from concourse.bass_utils import run_bass_kernel_spmd

    if "nc" not in _CACHE:
        _CACHE["nc"] = _build()
    nc = _CACHE["nc"]

    x = np.ascontiguousarray(inputs["x"], dtype=np.float32)
    adj = np.asarray(inputs["adj"])
    # pack adjacency bits into u16 words, k-stride 256 within each 4096-half:
    # adjp[n, s*256+w] bit j = adj[n, s*4096 + j*256 + w]
    A = (adj != 0).astype(np.uint16).reshape(N, 2, 16, 256)
    adjp = np.zeros((N, 2, 256), np.uint16)
    for j in range(16):
        adjp |= A[:, :, j, :] << j
    adjp = np.ascontiguousarray(adjp.reshape(N, 512))
    tar = np.asarray(inputs["tar_ei"]).astype(np.int32)

    def btile(b):
        return np.ascontiguousarray(
            np.asarray(b, dtype=np.float32).reshape(2, P).T)

    WN = ["xlin_w1", "xlin_w2", "xcn_w1", "xcn_w2", "xij_w", "lin_w1"]
    wall = np.zeros((P, 12 * C + 2), np.float32)
    for q, n in enumerate(WN):
        w = np.asarray(inputs[n], dtype=np.float32)  # [C, C]
        wall[:, (2 * q) * C:(2 * q + 1) * C] = w[:P, :]
        wall[:, (2 * q + 1) * C:(2 * q + 2) * C] = w[P:, :]
    lw2 = np.asarray(inputs["lin_w2"], dtype=np.float32).reshape(C, 1)
    wall[:, 12 * C] = lw2[:P, 0]
    wall[:, 12 * C + 1] = lw2[P:, 0]

    bnames = ["xlin_b1", "xlin_b2", "xcn_b1", "xcn_b2", "xij_b", "lin_b1"]
    ball = np.zeros((P, 2 * len(bnames) + 2), np.float32)
    for q, n in enumerate(bnames):
        ball[:, 2 * q:2 * q + 2] = btile(inputs[n])
    ball[:, 12] = np.asarray(inputs["beta"]).reshape(-1)[0]
    ball[:, 13] = np.asarray(inputs["lin_b2"]).reshape(-1)[0]

    common = {
        "x": x.astype(ml_dtypes.bfloat16),
        "xT": np.ascontiguousarray(x.T).astype(ml_dtypes.bfloat16),
        "adjp": adjp,
        "wall": np.ascontiguousarray(wall).astype(ml_dtypes.bfloat16),
        "ball": np.ascontiguousarray(ball),
    }

    in_maps = []
    for c in range(NCORES):
        m = dict(common)
        m["idx"] = np.ascontiguousarray(tar[:, c * EL:(c + 1) * EL])
        in_maps.append(m)

    res = run_bass_kernel_spmd(
        nc, in_maps, core_ids=list(range(NCORES)), trace=TRACE
    )
    global LAST_RESULT
    LAST_RESULT = res
    out = np.concatenate(
        [res.results[c]["out"].reshape(EL, 1) for c in range(NCORES)], axis=0
    )
    return out.astype(np.float32)


# revision 5
# speedup vs baseline: 1.5848x; 1.0503x over previous
"""CNLinkPredictor Trainium2 kernel.

Edge-sharded across 8 NeuronCores (1024 target edges each); x, adj, and the
MLP weights are replicated. Per core:
  A) h = x + MLP(x) in transposed layout: host supplies xT, stage A is
     matmul-only on PE (bf16, 512-node moving dim), fused bias+ReLU on the
     scalar engine, residual on DVE into wide [128, 2048] hT tiles, then one
     xbar DMA-transpose per (c-half, 4-group) writes h back to natural
     layout (column order (hh, kt, c2): h[node=kt*128+p, ch=hh*128+c2]).
  B) the adjacency is BIT-PACKED on the host into u16 words with k-stride
     256: adjp[n, s*256+w] bit j = adj[n, s*4096 + j*256 + w].  Per edge
     block (128 edges): one indirect-DMA gather per endpoint fetches the
     full 1024-byte packed row pair; a DVE bitwise-AND gives the packed
     common-neighbor mask; ONE 2-byte xbar transpose per block moves it
     k-major (64 KB instead of the 1 MB an unpacked transpose would be).
     Per (bit j, half s): ONE DVE bitvec op (and 1<<j, shl to bit 14)
     unpacks all 8 blocks at once into the bf16 bit pattern of 2.0 (the
     host pre-halves xcn_w1 to compensate; strided out AP groups columns
     into contiguous kt tiles), then - via bitcast - 8 matmuls accumulate
     xcnT[c, e] += h[kt]^T cn[kt] with h tiles stationary.  xcnT lands
     already transposed for stage C - no further transposes needed.
  C) xi*xj gathered per block (bf16 rows), DVE product, one packed xbar
     transpose per block; edge MLPs in transposed layout (512-edge
     groups).  The xij layer (independent of xcn) is emitted INTO the
     stage-B j-loop so PE/Act digest it during B; only the xcn-dependent
     chain (u1 -> u2 -> z -> v -> out) trails the last B matmul.

Scheduling rules this kernel follows (learned from TimelineSim traces):
  - Engine SEQs are in-order FIFOs and a waiting instruction parks on the
    queue head: all dependency-free DMAs (xT loads) are emitted before any
    dependent transpose on the same queue.
  - PSUM is 8 banks: stage A uses 4 (scoped), then stage B accumulators 4
    + stage C layer tiles 3 + output 1 coexist.

Hardware pitfalls this kernel works around:
  - This walrus build accepts at most ONE sync-wait per instruction
    (_apply_tile_patch splits the Tile tail drain; _split_multi_waits
    hoists extra waits onto same-engine NoOps).
  - Concurrent 4-byte DMA traffic corrupts in-flight 2-byte xbar
    DMA-transposes, so every steady-state transfer is <= 2 bytes/element
    (u16 packed adjacency, bf16 everything else); the few f32/int32 loads
    happen up front and the single f32 store happens after the last
    transpose.
  - The bitwise DVE path cannot cast dtypes (and rejects float dtypes),
    so the unpack writes u16 tiles holding bf16 bit patterns (0x4000=2.0)
    and the matmul reads them through AP.bitcast(bf16).
  - Indirect DMA consumes ONE index per out partition (the partition's
    whole span streams contiguously from the indexed row), so gathers are
    per-128-edge-block.
"""

import numpy as np
import ml_dtypes

N = 8192
C = 256
E = 8192
NCORES = 8
EL = E // NCORES          # edges per core
P = 128
NB = EL // P              # edge blocks per core (8)
NS = 2                    # k halves (4096 each)
NW = 256                  # u16 words per half per row
NJ = 16                   # bits per word
AGRP = 512                # stage-A node group
NGW = 4                   # stage-A transpose super-groups (4 groups each)
CGRP = 4                  # stage-C blocks per group (512 edges)

_CACHE = {}
TRACE = False
LAST_RESULT = None


def _apply_tile_patch():
    """Split the Tile tail-drain's multi-sem wait onto individual SP nops."""
    from concourse.tile import TileContext
    from concourse.vector_clock import ScopedClock

    if getattr(TileContext, "_drain_patched", False):
        return

    def _patched(self, tick_clock, wait_clock):
        nc = self.nc
        collector = nc.sync.nop()
        wait_clock.add_sem_waits(
            collector.ins, ScopedClock({None: tick_clock.global_clock})
        )
        si = collector.ins.sync_info
        waits = list(si.on_wait) if si is not None and si.on_wait else []
        if si is not None and len(waits) > 1:
            name_to_handle = {h.name: h for h in self.sems.allocated().values()}
            si.on_wait = [waits[0]]
            for w in waits[1:]:
                op = {
                    "sem-ge-imm": "sem-ge",
                    "sem-eq-imm": "sem-eq",
                    "sem-le-imm": "sem-le",
                }.get(str(w.wait_mode), "sem-ge")
                nc.sync.nop().wait_op(name_to_handle[w.ant_name], w.wait_value, op)
        nc.sync.drain()
        nc.all_engine_barrier()
        assert self.sems is not None
        popped = nc._tile_sem_poison_stack.pop()
        assert popped is self._sem_poison
        nc.clear_and_free_semaphores(list(self.sems.allocated().values()))
        nc.all_engine_barrier()

    TileContext._drain_and_barrier = _patched
    TileContext._drain_patched = True


def _split_multi_waits(nc):
    """Hoist extra sync-waits onto same-engine NoOps (sequential waits ==
    ANDed waits); this walrus build allows one wait per instruction."""
    import concourse.mybir as mybir

    cnt = 0
    for fn in nc.m.functions:
        for bb in fn.blocks:
            out = []
            for inst in bb.instructions:
                si = getattr(inst, "sync_info", None)
                waits = list(si.on_wait) if si is not None and si.on_wait else []
                if len(waits) > 1:
                    for w in waits[:-1]:
                        nop = mybir.InstNoOp(name=f"ws-{cnt}", ins=[], outs=[])
                        cnt += 1
                        nop.engine = inst.engine
                        nop.sync_info = mybir.SyncInfo(on_wait=[w], on_update=[])
                        out.append(nop)
                    si.on_wait = [waits[-1]]
                out.append(inst)
            bb.instructions = out
    return nc


def _build(split_waits=True):
    import concourse.bass as bass
    import concourse.mybir as mybir
    from concourse.tile import TileContext

    _apply_tile_patch()

    f32 = mybir.dt.float32
    bf16 = mybir.dt.bfloat16
    u16 = mybir.dt.uint16
    i32 = mybir.dt.int32
    Relu = mybir.ActivationFunctionType.Relu
    Ident = mybir.ActivationFunctionType.Identity
    MUL = mybir.AluOpType.mult
    ADD = mybir.AluOpType.add
    ANDB = mybir.AluOpType.bitwise_and
    SHR = mybir.AluOpType.logical_shift_right
    SHL = mybir.AluOpType.logical_shift_left
    MAX = mybir.AluOpType.max

    nc = bass.Bass(num_swdge_queues=4)

    xT_d = nc.dram_tensor("xT", [C, N], bf16, kind="ExternalInput")
    x_d = nc.dram_tensor("x", [N, C], bf16, kind="ExternalInput")
    adjp_d = nc.dram_tensor("adjp", [N, NS * NW], u16, kind="ExternalInput")
    idx_d = nc.dram_tensor("idx", [2, EL], i32, kind="ExternalInput")
    # all bf16 matmul weights in one load: 6 [C,C] ws + lin_w2 padded to 2 cols
    WN = ["xlin_w1", "xlin_w2", "xcn_w1", "xcn_w2", "xij_w", "lin_w1"]
    wall_d = nc.dram_tensor("wall", [P, 12 * C + 2], bf16, kind="ExternalInput")
    # f32 consts: 6 bias pairs + beta + lin_b2
    bnames = ["xlin_b1", "xlin_b2", "xcn_b1", "xcn_b2", "xij_b", "lin_b1"]
    ball_d = nc.dram_tensor("ball", [P, 2 * len(bnames) + 2], f32,
                            kind="ExternalInput")
    out_d = nc.dram_tensor("out", [1, EL], f32, kind="ExternalOutput")

    _swq = [0]

    def _rr(inst):
        q = _swq[0] % 4
        _swq[0] += 1
        if q:
            inst.ins.queue = f"qPoolDynamic{q}"
        return inst

    with TileContext(nc) as tc:
        with (
            tc.tile_pool(name="const", bufs=1) as pK,
            tc.tile_pool(name="hpool", bufs=1) as pH,
            tc.tile_pool(name="adj", bufs=4) as pAdj,
            tc.tile_pool(name="cnp", bufs=4) as pCn,
            tc.tile_pool(name="unp", bufs=4) as pU,
            tc.tile_pool(name="edge", bufs=1) as pE,
            tc.tile_pool(name="stC", bufs=2) as pC,
        ):
            # ---- constants ----
            idx_sb = pK.tile([P, 2 * NB], i32, tag="idx_sb", name="idx_sb")
            nc.sync.dma_start(
                out=idx_sb[:].rearrange("p (t b) -> p t b", t=2),
                in_=idx_d[:, :].rearrange("t (b p) -> p t b", p=P),
            )
            ii = [idx_sb[:, b:b + 1] for b in range(NB)]
            jj = [idx_sb[:, NB + b:NB + b + 1] for b in range(NB)]

            wall = pK.tile([P, 12 * C + 2], bf16, tag="wall", name="wall")
            nc.sync.dma_start(out=wall[:, :4 * C], in_=wall_d[:, :4 * C])
            w_sb = {}
            for q, n in enumerate(WN):
                w_sb[n] = [wall[:, (2 * q) * C:(2 * q + 1) * C],
                           wall[:, (2 * q + 1) * C:(2 * q + 2) * C]]
            lw2_sb = [wall[:, 12 * C:12 * C + 1], wall[:, 12 * C + 1:12 * C + 2]]

            xTw = {}
            for h in range(2):
                t = pK.tile([P, 4 * AGRP], bf16, tag=f"xTw0_{h}",
                            name=f"xTw0_{h}")
                nc.sync.dma_start(out=t[:], in_=xT_d[h * P:(h + 1) * P,
                                                     0:4 * AGRP])
                xTw[(0, h)] = t
            nc.sync.dma_start(out=wall[:, 4 * C:], in_=wall_d[:, 4 * C:])
            ball = pK.tile([P, 2 * len(bnames) + 2], f32, tag="ball", name="ball")
            nc.sync.dma_start(out=ball[:], in_=ball_d[:, :])
            b_sb = {n: ball[:, 2 * q:2 * q + 2] for q, n in enumerate(bnames)}
            beta_sb = ball[:, 12:13]
            lb2_sb = ball[:, 13:14]

            out_row = pK.tile([1, EL], f32, tag="out_row", name="out_row")
            # natural-layout h: column = hh*N + kt*128 + c2 holds
            # h[node = kt*128 + p, channel = hh*128 + c2]
            h_all = pH.tile([P, 2 * N], bf16, tag="h_all", name="h_all")
            h_view = h_all[:].rearrange("p (hh kt c) -> p hh kt c", hh=2, c=P)

            # ---- remaining stage-A xT loads: dependency-free on SP.SEQ ----
            for gw in range(1, NGW):
                for h in range(2):
                    t = pK.tile([P, 4 * AGRP], bf16, tag=f"xTw{gw}_{h}",
                                name=f"xTw{gw}_{h}")
                    nc.sync.dma_start(
                        out=t[:],
                        in_=xT_d[h * P:(h + 1) * P,
                                 gw * 4 * AGRP:(gw + 1) * 4 * AGRP],
                    )
                    xTw[(gw, h)] = t

            # ---- stage B gathers + AND + packed transposes ----
            # T_all[p, b, 2s+wc, e] = word (s, wc*128+p) of edge (b,e)'s cn mask
            T_all = pH.tile([P, NB * 4 * P], u16, tag="T_all", name="T_all")
            T_view = T_all[:].rearrange("p (b ch e) -> p b ch e", b=NB, e=P)
            for b in range(NB):
                ai = pAdj.tile([P, NS * NW], u16, tag="ai", name=f"ai{b}")
                _rr(nc.gpsimd.indirect_dma_start(
                    out=ai[:], out_offset=None, in_=adjp_d[:, :],
                    in_offset=bass.IndirectOffsetOnAxis(ap=ii[b][:, :1], axis=0),
                ))
                aj = pAdj.tile([P, NS * NW], u16, tag="aj", name=f"aj{b}")
                _rr(nc.gpsimd.indirect_dma_start(
                    out=aj[:], out_offset=None, in_=adjp_d[:, :],
                    in_offset=bass.IndirectOffsetOnAxis(ap=jj[b][:, :1], axis=0),
                ))
                cnp = pCn.tile([P, NS * NW], u16, tag="cnp", name=f"cnp{b}")
                nc.vector.tensor_tensor(out=cnp[:], in0=ai[:], in1=aj[:], op=ANDB)
                nc.sync.dma_start_transpose(out=T_view[:, b, :, :], in_=cnp[:])

            # ---- stage C gathers (Pool queue, after adjacency gathers) ----
            exi, exj = [], []
            for b in range(NB):
                xi = pE.tile([P, C], bf16, tag=f"xi{b}", name=f"xi{b}")
                _rr(nc.gpsimd.indirect_dma_start(
                    out=xi[:], out_offset=None, in_=x_d[:, :],
                    in_offset=bass.IndirectOffsetOnAxis(ap=ii[b][:, :1], axis=0),
                ))
                xj = pE.tile([P, C], bf16, tag=f"xj{b}", name=f"xj{b}")
                _rr(nc.gpsimd.indirect_dma_start(
                    out=xj[:], out_offset=None, in_=x_d[:, :],
                    in_offset=bass.IndirectOffsetOnAxis(ap=jj[b][:, :1], axis=0),
                ))
                exi.append(xi)
                exj.append(xj)

            # ---- stage A ----
            with tc.tile_pool(name="stA", bufs=3) as pA, \
                 tc.tile_pool(name="hT", bufs=2) as pHT, \
                 tc.tile_pool(name="psA", bufs=8, space="PSUM") as psA:
                hTw = {}
                for g in range(4 * NGW):
                    gw, gl = g // 4, g % 4
                    if gl == 0:
                        for h in range(2):
                            hTw[h] = pHT.tile([P, 4 * AGRP], bf16,
                                              tag=f"hTw{h}", name=f"hTw{gw}_{h}")
                    xT = [xTw[(gw, h)][:, gl * AGRP:(gl + 1) * AGRP]
                          for h in range(2)]
                    y1T = []
                    for h in range(2):
                        ps = psA.tile([P, AGRP], f32, tag="psmm",
                                      name=f"psA1_{g}{h}")
                        nc.tensor.matmul(
                            ps[:], w_sb["xlin_w1"][0][:, h * P:(h + 1) * P],
                            xT[0], start=True, stop=False,
                        )
                        nc.tensor.matmul(
                            ps[:], w_sb["xlin_w1"][1][:, h * P:(h + 1) * P],
                            xT[1], start=False, stop=True,
                        )
                        t = pA.tile([P, AGRP], bf16, tag=f"y1T{h}",
                                    name=f"y1T{h}_{g}")
                        nc.scalar.activation(
                            t[:], ps[:], Relu, bias=b_sb["xlin_b1"][:, h:h + 1]
                        )
                        y1T.append(t)
                    for h in range(2):
                        ps = psA.tile([P, AGRP], f32, tag="psmm",
                                      name=f"psA2_{g}{h}")
                        nc.tensor.matmul(
                            ps[:], w_sb["xlin_w2"][0][:, h * P:(h + 1) * P],
                            y1T[0][:], start=True, stop=False,
                        )
                        nc.tensor.matmul(
                            ps[:], w_sb["xlin_w2"][1][:, h * P:(h + 1) * P],
                            y1T[1][:], start=False, stop=True,
                        )
                        y2 = pA.tile([P, AGRP], bf16, tag="y2T", name=f"y2T{h}_{g}")
                        if h == 0:
                            nc.scalar.activation(
                                y2[:], ps[:], Relu,
                                bias=b_sb["xlin_b2"][:, h:h + 1]
                            )
                        else:
                            nc.vector.tensor_scalar(
                                out=y2[:], in0=ps[:],
                                scalar1=b_sb["xlin_b2"][:, h:h + 1],
                                scalar2=0.0, op0=ADD, op1=MAX,
                            )
                        nc.vector.tensor_tensor(
                            out=hTw[h][:, gl * AGRP:(gl + 1) * AGRP],
                            in0=xT[h], in1=y2[:], op=ADD,
                        )
                    if gl == 3:
                        for h in range(2):
                            nc.sync.dma_start_transpose(
                                out=h_view[:, h, gw * 16:(gw + 1) * 16, :],
                                in_=hTw[h][:],
                            )

            # ---- edge products + transposes (DVE/SP, ready mid j-loop) ----
            prodT_all = pH.tile([P, NB * 2 * P], bf16, tag="prodT",
                                name="prodT_all")
            prodT_v = prodT_all[:].rearrange("p (b ch e) -> p b ch e",
                                             b=NB, e=P)
            for b in range(NB):
                pt = pE.tile([P, C], bf16, tag=f"prod{b}", name=f"prod{b}")
                nc.vector.tensor_tensor(
                    out=pt[:], in0=exi[b][:], in1=exj[b][:], op=MUL
                )
                nc.sync.dma_start_transpose(out=prodT_v[:, b, :, :], in_=pt[:])

            # ---- stage B j-loop + interleaved stage-C xij layer ----
            with tc.tile_pool(name="psB", bufs=1, space="PSUM") as psB, \
                 tc.tile_pool(name="psC", bufs=3, space="PSUM") as psC, \
                 tc.tile_pool(name="psO", bufs=1, space="PSUM") as psO:
                ps_x = {}
                for hh in range(2):
                    for eh in range(2):
                        ps_x[(hh, eh)] = psB.tile(
                            [P, EL // 2], f32, tag=f"psx{hh}{eh}",
                            name=f"psx{hh}{eh}")

                def mlp_layer(grp, rhs_pair, wname, bname, outtag):
                    W = 512
                    outs = []
                    for h in range(2):
                        ps = psC.tile([P, W], f32, tag="psc",
                                      name=f"psc_{grp}_{outtag}{h}")
                        nc.tensor.matmul(
                            ps[:], w_sb[wname][0][:, h * P:(h + 1) * P],
                            rhs_pair[0], start=True, stop=False,
                        )
                        nc.tensor.matmul(
                            ps[:], w_sb[wname][1][:, h * P:(h + 1) * P],
                            rhs_pair[1], start=False, stop=True,
                        )
                        t = pC.tile([P, W], bf16, tag=f"{outtag}{h}",
                                    name=f"{outtag}{h}_{grp}")
                        nc.scalar.activation(
                            t[:], ps[:], Relu, bias=b_sb[bname][:, h:h + 1]
                        )
                        outs.append(t)
                    return outs

                xijT = {}
                for j in range(NJ):
                    for s in range(2):
                        uj = pU.tile([P, 2 * EL], u16, tag="uj",
                                     name=f"uj{j}_{s}")
                        # single bitvec op: isolate bit j and move it to bit
                        # 14 = the bf16 bit pattern of 2.0 (host pre-halves
                        # xcn_w1 to compensate).  out col = wc*1024+b*128+e;
                        # in iterates (b, wc, e) = T_all's (b, ch=2s+wc, e)
                        ts_kw = dict(
                            out=uj[:].rearrange("p (wc b e) -> p b wc e",
                                                wc=2, e=P),
                            in0=T_view[:, :, 2 * s:2 * s + 2, :],
                            scalar1=1 << j,
                        )
                        if j < 14:
                            nc.vector.tensor_scalar(
                                scalar2=14 - j, op0=ANDB, op1=SHL, **ts_kw)
                        elif j == 14:
                            nc.vector.tensor_scalar(
                                scalar2=None, op0=ANDB, **ts_kw)
                        else:
                            nc.vector.tensor_scalar(
                                scalar2=1, op0=ANDB, op1=SHR, **ts_kw)
                        cj = uj[:].bitcast(bf16)
                        for wc in range(2):
                            kt = s * 32 + 2 * j + wc
                            for hh in range(2):
                                for eh in range(2):
                                    nc.tensor.matmul(
                                        ps_x[(hh, eh)][:],
                                        h_view[:, hh, kt, :],
                                        cj[:, wc * EL + eh * 512:
                                           wc * EL + (eh + 1) * 512],
                                        start=(j == 0 and s == 0 and wc == 0),
                                        stop=(j == NJ - 1 and s == 1
                                              and wc == 1),
                                    )
                    if j == 9:
                        # PE/Act slack inside the j-loop: xij layer for both
                        # 512-edge groups (depends only on prodT)
                        for grp in range(EL // 512):
                            xijT[grp] = mlp_layer(
                                grp,
                                [prodT_v[:, grp * CGRP:(grp + 1) * CGRP, ch, :]
                                 for ch in range(2)],
                                "xij_w", "xij_b", "xijT")

                xcn_sb = pH.tile([P, 2 * EL], bf16, tag="xcn", name="xcn_sb")
                for hh in range(2):
                    for eh in range(2):
                        nc.vector.tensor_copy(
                            xcn_sb[:, hh * EL + eh * 512:
                                   hh * EL + (eh + 1) * 512],
                            ps_x[(hh, eh)][:])

                # ---- stage C xcn-dependent chain ----
                for grp in range(EL // 512):
                    W = 512
                    xcn_rhs = [
                        xcn_sb[:, ch * EL + grp * W:ch * EL + (grp + 1) * W]
                        for ch in range(2)
                    ]
                    u1T = mlp_layer(grp, xcn_rhs, "xcn_w1", "xcn_b1", "u1T")
                    u2T = mlp_layer(grp, [u1T[0][:], u1T[1][:]],
                                    "xcn_w2", "xcn_b2", "u2T")
                    zT = []
                    for h in range(2):
                        zb = pC.tile([P, W], bf16, tag=f"zb{h}",
                                     name=f"zb{h}_{grp}")
                        nc.vector.tensor_tensor(
                            out=zb[:], in0=u2T[h][:],
                            in1=beta_sb.to_broadcast([P, W]), op=MUL,
                        )
                        zt = pC.tile([P, W], bf16, tag=f"zT{h}",
                                     name=f"zT{h}_{grp}")
                        nc.vector.tensor_tensor(
                            out=zt[:], in0=zb[:], in1=xijT[grp][h][:], op=ADD
                        )
                        zT.append(zt)
                    vT = mlp_layer(grp, [zT[0][:], zT[1][:]],
                                   "lin_w1", "lin_b1", "vT")
                    pso = psO.tile([1, W], f32, tag="pso", name=f"pso{grp}")
                    nc.tensor.matmul(
                        pso[:], lw2_sb[0][:], vT[0][:], start=True, stop=False
                    )
                    nc.tensor.matmul(
                        pso[:], lw2_sb[1][:], vT[1][:], start=False, stop=True
                    )
                    nc.scalar.activation(
                        out_row[0:1, grp * W:(grp + 1) * W], pso[:],
                        Ident, bias=lb2_sb[0:1, 0:1],
                    )

            nc.sync.dma_start(out=out_d[:, :], in_=out_row[0:1, :])

    return _split_multi_waits(nc) if split_waits else nc


def kernel(**inputs):
    from concourse.bass_utils import run_bass_kernel_spmd

    if "nc" not in _CACHE:
        _CACHE["nc"] = _build()
    nc = _CACHE["nc"]

    x = np.ascontiguousarray(inputs["x"], dtype=np.float32)
    adj = np.asarray(inputs["adj"])
    # pack adjacency bits into u16 words, k-stride 256 within each 4096-half:
    # adjp[n, s*256+w] bit j = adj[n, s*4096 + j*256 + w]
    A = (adj != 0).astype(np.uint16).reshape(N, 2, 16, 256)
    adjp = np.zeros((N, 2, 256), np.uint16)
    for j in range(16):
        adjp |= A[:, :, j, :] << j
    adjp = np.ascontiguousarray(adjp.reshape(N, 512))
    tar = np.asarray(inputs["tar_ei"]).astype(np.int32)

    def btile(b):
        return np.ascontiguousarray(
            np.asarray(b, dtype=np.float32).reshape(2, P).T)

    WN = ["xlin_w1", "xlin_w2", "xcn_w1", "xcn_w2", "xij_w", "lin_w1"]
    wall = np.zeros((P, 12 * C + 2), np.float32)
    for q, n in enumerate(WN):
        w = np.asarray(inputs[n], dtype=np.float32)  # [C, C]
        if n == "xcn_w1":
            w = w * 0.5  # stage-B cn values are 2.0, not 1.0
        wall[:, (2 * q) * C:(2 * q + 1) * C] = w[:P, :]
        wall[:, (2 * q + 1) * C:(2 * q + 2) * C] = w[P:, :]
    lw2 = np.asarray(inputs["lin_w2"], dtype=np.float32).reshape(C, 1)
    wall[:, 12 * C] = lw2[:P, 0]
    wall[:, 12 * C + 1] = lw2[P:, 0]

    bnames = ["xlin_b1", "xlin_b2", "xcn_b1", "xcn_b2", "xij_b", "lin_b1"]
    ball = np.zeros((P, 2 * len(bnames) + 2), np.float32)
    for q, n in enumerate(bnames):
        ball[:, 2 * q:2 * q + 2] = btile(inputs[n])
    ball[:, 12] = np.asarray(inputs["beta"]).reshape(-1)[0]
    ball[:, 13] = np.asarray(inputs["lin_b2"]).reshape(-1)[0]

    common = {
        "x": x.astype(ml_dtypes.bfloat16),
        "xT": np.ascontiguousarray(x.T).astype(ml_dtypes.bfloat16),
        "adjp": adjp,
        "wall": np.ascontiguousarray(wall).astype(ml_dtypes.bfloat16),
        "ball": np.ascontiguousarray(ball),
    }

    in_maps = []
    for c in range(NCORES):
        m = dict(common)
        m["idx"] = np.ascontiguousarray(tar[:, c * EL:(c + 1) * EL])
        in_maps.append(m)

    res = run_bass_kernel_spmd(
        nc, in_maps, core_ids=list(range(NCORES)), trace=TRACE
    )
    global LAST_RESULT
    LAST_RESULT = res
    out = np.concatenate(
        [res.results[c]["out"].reshape(EL, 1) for c in range(NCORES)], axis=0
    )
    return out.astype(np.float32)


# revision 6
# speedup vs baseline: 1.5895x; 1.0029x over previous
"""CNLinkPredictor Trainium2 kernel.

Edge-sharded across 8 NeuronCores (1024 target edges each); x, adj, and the
MLP weights are replicated. Per core:
  A) h = x + MLP(x) in transposed layout: host supplies xT, stage A is
     matmul-only on PE (bf16, 512-node moving dim), fused bias+ReLU on the
     scalar engine, residual on DVE into wide [128, 2048] hT tiles, then one
     xbar DMA-transpose per (c-half, 4-group) writes h back to natural
     layout (column order (hh, kt, c2): h[node=kt*128+p, ch=hh*128+c2]).
  B) the adjacency is BIT-PACKED on the host into u16 words with k-stride
     256: adjp[n, s*256+w] bit j = adj[n, s*4096 + j*256 + w].  Per edge
     block (128 edges): one indirect-DMA gather per endpoint fetches the
     full 1024-byte packed row pair; a DVE bitwise-AND gives the packed
     common-neighbor mask; ONE 2-byte xbar transpose per block moves it
     k-major (64 KB instead of the 1 MB an unpacked transpose would be).
     Per (bit j, half s): ONE DVE bitvec op (and 1<<j, shl to bit 14)
     unpacks all 8 blocks at once into the bf16 bit pattern of 2.0 (the
     host pre-halves xcn_w1 to compensate; strided out AP groups columns
     into contiguous kt tiles), then - via bitcast - 8 matmuls accumulate
     xcnT[c, e] += h[kt]^T cn[kt] with h tiles stationary.  xcnT lands
     already transposed for stage C - no further transposes needed.
  C) xi*xj gathered per block (bf16 rows), DVE product, one packed xbar
     transpose per block; edge MLPs in transposed layout (512-edge
     groups).  The xij layer (independent of xcn) is emitted INTO the
     stage-B j-loop so PE/Act digest it during B; only the xcn-dependent
     chain (u1 -> u2 -> z -> v -> out) trails the last B matmul.

Scheduling rules this kernel follows (learned from TimelineSim traces):
  - Engine SEQs are in-order FIFOs and a waiting instruction parks on the
    queue head: all dependency-free DMAs (xT loads) are emitted before any
    dependent transpose on the same queue.
  - PSUM is 8 banks: stage A uses 4 (scoped), then stage B accumulators 4
    + stage C layer tiles 3 + output 1 coexist.

Hardware pitfalls this kernel works around:
  - This walrus build accepts at most ONE sync-wait per instruction
    (_apply_tile_patch splits the Tile tail drain; _split_multi_waits
    hoists extra waits onto same-engine NoOps).
  - Concurrent 4-byte DMA traffic corrupts in-flight 2-byte xbar
    DMA-transposes, so every steady-state transfer is <= 2 bytes/element
    (u16 packed adjacency, bf16 everything else); the few f32/int32 loads
    happen up front and the single f32 store happens after the last
    transpose.
  - The bitwise DVE path cannot cast dtypes (and rejects float dtypes),
    so the unpack writes u16 tiles holding bf16 bit patterns (0x4000=2.0)
    and the matmul reads them through AP.bitcast(bf16).
  - Indirect DMA consumes ONE index per out partition (the partition's
    whole span streams contiguously from the indexed row), so gathers are
    per-128-edge-block.
"""

import numpy as np
import ml_dtypes

N = 8192
C = 256
E = 8192
NCORES = 8
EL = E // NCORES          # edges per core
P = 128
NB = EL // P              # edge blocks per core (8)
NS = 2                    # k halves (4096 each)
NW = 256                  # u16 words per half per row
NJ = 16                   # bits per word
AGRP = 512                # stage-A node group
NGW = 4                   # stage-A transpose super-groups (4 groups each)
CGRP = 4                  # stage-C blocks per group (512 edges)

_CACHE = {}
TRACE = False
LAST_RESULT = None


def _apply_tile_patch():
    """Split the Tile tail-drain's multi-sem wait onto individual SP nops."""
    from concourse.tile import TileContext
    from concourse.vector_clock import ScopedClock

    if getattr(TileContext, "_drain_patched", False):
        return

    def _patched(self, tick_clock, wait_clock):
        nc = self.nc
        collector = nc.sync.nop()
        wait_clock.add_sem_waits(
            collector.ins, ScopedClock({None: tick_clock.global_clock})
        )
        si = collector.ins.sync_info
        waits = list(si.on_wait) if si is not None and si.on_wait else []
        if si is not None and len(waits) > 1:
            name_to_handle = {h.name: h for h in self.sems.allocated().values()}
            si.on_wait = [waits[0]]
            for w in waits[1:]:
                op = {
                    "sem-ge-imm": "sem-ge",
                    "sem-eq-imm": "sem-eq",
                    "sem-le-imm": "sem-le",
                }.get(str(w.wait_mode), "sem-ge")
                nc.sync.nop().wait_op(name_to_handle[w.ant_name], w.wait_value, op)
        nc.sync.drain()
        nc.all_engine_barrier()
        assert self.sems is not None
        popped = nc._tile_sem_poison_stack.pop()
        assert popped is self._sem_poison
        nc.clear_and_free_semaphores(list(self.sems.allocated().values()))
        nc.all_engine_barrier()

    TileContext._drain_and_barrier = _patched
    TileContext._drain_patched = True


def _split_multi_waits(nc):
    """Hoist extra sync-waits onto same-engine NoOps (sequential waits ==
    ANDed waits); this walrus build allows one wait per instruction."""
    import concourse.mybir as mybir

    cnt = 0
    for fn in nc.m.functions:
        for bb in fn.blocks:
            out = []
            for inst in bb.instructions:
                si = getattr(inst, "sync_info", None)
                waits = list(si.on_wait) if si is not None and si.on_wait else []
                if len(waits) > 1:
                    for w in waits[:-1]:
                        nop = mybir.InstNoOp(name=f"ws-{cnt}", ins=[], outs=[])
                        cnt += 1
                        nop.engine = inst.engine
                        nop.sync_info = mybir.SyncInfo(on_wait=[w], on_update=[])
                        out.append(nop)
                    si.on_wait = [waits[-1]]
                out.append(inst)
            bb.instructions = out
    return nc


def _build(split_waits=True):
    import concourse.bass as bass
    import concourse.mybir as mybir
    from concourse.tile import TileContext

    _apply_tile_patch()

    f32 = mybir.dt.float32
    bf16 = mybir.dt.bfloat16
    u16 = mybir.dt.uint16
    i32 = mybir.dt.int32
    Relu = mybir.ActivationFunctionType.Relu
    Ident = mybir.ActivationFunctionType.Identity
    MUL = mybir.AluOpType.mult
    ADD = mybir.AluOpType.add
    ANDB = mybir.AluOpType.bitwise_and
    SHR = mybir.AluOpType.logical_shift_right
    SHL = mybir.AluOpType.logical_shift_left
    MAX = mybir.AluOpType.max

    nc = bass.Bass(num_swdge_queues=4)

    xT_d = nc.dram_tensor("xT", [C, N], bf16, kind="ExternalInput")
    x_d = nc.dram_tensor("x", [N, C], bf16, kind="ExternalInput")
    adjp_d = nc.dram_tensor("adjp", [N, NS * NW], u16, kind="ExternalInput")
    idx_d = nc.dram_tensor("idx", [2, EL], i32, kind="ExternalInput")
    # all bf16 matmul weights in one load: 6 [C,C] ws + lin_w2 padded to 2 cols
    WN = ["xlin_w1", "xlin_w2", "xcn_w1", "xcn_w2", "xij_w", "lin_w1"]
    wall_d = nc.dram_tensor("wall", [P, 12 * C + 2], bf16, kind="ExternalInput")
    # f32 consts: 6 bias pairs + beta + lin_b2
    bnames = ["xlin_b1", "xlin_b2", "xcn_b1", "xcn_b2", "xij_b", "lin_b1"]
    ball_d = nc.dram_tensor("ball", [P, 2 * len(bnames) + 2], f32,
                            kind="ExternalInput")
    out_d = nc.dram_tensor("out", [1, EL], f32, kind="ExternalOutput")

    _swq = [0]

    def _rr(inst):
        q = _swq[0] % 4
        _swq[0] += 1
        if q:
            inst.ins.queue = f"qPoolDynamic{q}"
        return inst

    with TileContext(nc) as tc:
        with (
            tc.tile_pool(name="const", bufs=1) as pK,
            tc.tile_pool(name="hpool", bufs=1) as pH,
            tc.tile_pool(name="adj", bufs=4) as pAdj,
            tc.tile_pool(name="unp", bufs=4) as pU,
            tc.tile_pool(name="edge", bufs=1) as pE,
            tc.tile_pool(name="stC", bufs=2) as pC,
        ):
            # ---- constants ----
            idx_sb = pK.tile([P, 2 * NB], i32, tag="idx_sb", name="idx_sb")
            nc.sync.dma_start(
                out=idx_sb[:].rearrange("p (t b) -> p t b", t=2),
                in_=idx_d[:, :].rearrange("t (b p) -> p t b", p=P),
            )
            ii = [idx_sb[:, b:b + 1] for b in range(NB)]
            jj = [idx_sb[:, NB + b:NB + b + 1] for b in range(NB)]

            wall = pK.tile([P, 12 * C + 2], bf16, tag="wall", name="wall")
            nc.sync.dma_start(out=wall[:, :4 * C], in_=wall_d[:, :4 * C])
            w_sb = {}
            for q, n in enumerate(WN):
                w_sb[n] = [wall[:, (2 * q) * C:(2 * q + 1) * C],
                           wall[:, (2 * q + 1) * C:(2 * q + 2) * C]]
            lw2_sb = [wall[:, 12 * C:12 * C + 1], wall[:, 12 * C + 1:12 * C + 2]]

            xTw = {}
            for h in range(2):
                t = pK.tile([P, 4 * AGRP], bf16, tag=f"xTw0_{h}",
                            name=f"xTw0_{h}")
                nc.sync.dma_start(out=t[:], in_=xT_d[h * P:(h + 1) * P,
                                                     0:4 * AGRP])
                xTw[(0, h)] = t
            nc.sync.dma_start(out=wall[:, 4 * C:], in_=wall_d[:, 4 * C:])
            ball = pK.tile([P, 2 * len(bnames) + 2], f32, tag="ball", name="ball")
            nc.sync.dma_start(out=ball[:], in_=ball_d[:, :])
            b_sb = {n: ball[:, 2 * q:2 * q + 2] for q, n in enumerate(bnames)}
            beta_sb = ball[:, 12:13]
            lb2_sb = ball[:, 13:14]

            out_row = pK.tile([1, EL], f32, tag="out_row", name="out_row")
            # natural-layout h: column = hh*N + kt*128 + c2 holds
            # h[node = kt*128 + p, channel = hh*128 + c2]
            h_all = pH.tile([P, 2 * N], bf16, tag="h_all", name="h_all")
            h_view = h_all[:].rearrange("p (hh kt c) -> p hh kt c", hh=2, c=P)

            # ---- remaining stage-A xT loads: dependency-free on SP.SEQ ----
            for gw in range(1, NGW):
                for h in range(2):
                    t = pK.tile([P, 4 * AGRP], bf16, tag=f"xTw{gw}_{h}",
                                name=f"xTw{gw}_{h}")
                    nc.sync.dma_start(
                        out=t[:],
                        in_=xT_d[h * P:(h + 1) * P,
                                 gw * 4 * AGRP:(gw + 1) * 4 * AGRP],
                    )
                    xTw[(gw, h)] = t

            # ---- stage B gathers + packed transposes (AND after transpose:
            # the transposes then wait only on their own gather, keeping the
            # DMA queues flowing) ----
            # T*[p, b, 2s+wc, e] = word (s, wc*128+p) of edge (b,e)'s row
            T_all = pH.tile([P, NB * 4 * P], u16, tag="T_all", name="T_all")
            T_view = T_all[:].rearrange("p (b ch e) -> p b ch e", b=NB, e=P)
            Ti_all = pH.tile([P, NB * 4 * P], u16, tag="Ti_all", name="Ti_all")
            Ti_view = Ti_all[:].rearrange("p (b ch e) -> p b ch e", b=NB, e=P)
            Tj_all = pH.tile([P, NB * 4 * P], u16, tag="Tj_all", name="Tj_all")
            Tj_view = Tj_all[:].rearrange("p (b ch e) -> p b ch e", b=NB, e=P)
            for b in range(NB):
                ai = pAdj.tile([P, NS * NW], u16, tag="ai", name=f"ai{b}")
                _rr(nc.gpsimd.indirect_dma_start(
                    out=ai[:], out_offset=None, in_=adjp_d[:, :],
                    in_offset=bass.IndirectOffsetOnAxis(ap=ii[b][:, :1], axis=0),
                ))
                aj = pAdj.tile([P, NS * NW], u16, tag="aj", name=f"aj{b}")
                _rr(nc.gpsimd.indirect_dma_start(
                    out=aj[:], out_offset=None, in_=adjp_d[:, :],
                    in_offset=bass.IndirectOffsetOnAxis(ap=jj[b][:, :1], axis=0),
                ))
                nc.sync.dma_start_transpose(out=Ti_view[:, b, :, :], in_=ai[:])
                nc.sync.dma_start_transpose(out=Tj_view[:, b, :, :], in_=aj[:])
            for b in range(NB):
                nc.vector.tensor_tensor(
                    out=T_view[:, b, :, :], in0=Ti_view[:, b, :, :],
                    in1=Tj_view[:, b, :, :], op=ANDB)

            # ---- stage C gathers (Pool queue, after adjacency gathers) ----
            exi, exj = [], []
            for b in range(NB):
                xi = pE.tile([P, C], bf16, tag=f"xi{b}", name=f"xi{b}")
                _rr(nc.gpsimd.indirect_dma_start(
                    out=xi[:], out_offset=None, in_=x_d[:, :],
                    in_offset=bass.IndirectOffsetOnAxis(ap=ii[b][:, :1], axis=0),
                ))
                xj = pE.tile([P, C], bf16, tag=f"xj{b}", name=f"xj{b}")
                _rr(nc.gpsimd.indirect_dma_start(
                    out=xj[:], out_offset=None, in_=x_d[:, :],
                    in_offset=bass.IndirectOffsetOnAxis(ap=jj[b][:, :1], axis=0),
                ))
                exi.append(xi)
                exj.append(xj)

            # ---- stage A ----
            with tc.tile_pool(name="stA", bufs=3) as pA, \
                 tc.tile_pool(name="hT", bufs=2) as pHT, \
                 tc.tile_pool(name="psA", bufs=8, space="PSUM") as psA:
                hTw = {}
                for g in range(4 * NGW):
                    gw, gl = g // 4, g % 4
                    if gl == 0:
                        for h in range(2):
                            hTw[h] = pHT.tile([P, 4 * AGRP], bf16,
                                              tag=f"hTw{h}", name=f"hTw{gw}_{h}")
                    xT = [xTw[(gw, h)][:, gl * AGRP:(gl + 1) * AGRP]
                          for h in range(2)]
                    y1T = []
                    for h in range(2):
                        ps = psA.tile([P, AGRP], f32, tag="psmm",
                                      name=f"psA1_{g}{h}")
                        nc.tensor.matmul(
                            ps[:], w_sb["xlin_w1"][0][:, h * P:(h + 1) * P],
                            xT[0], start=True, stop=False,
                        )
                        nc.tensor.matmul(
                            ps[:], w_sb["xlin_w1"][1][:, h * P:(h + 1) * P],
                            xT[1], start=False, stop=True,
                        )
                        t = pA.tile([P, AGRP], bf16, tag=f"y1T{h}",
                                    name=f"y1T{h}_{g}")
                        nc.scalar.activation(
                            t[:], ps[:], Relu, bias=b_sb["xlin_b1"][:, h:h + 1]
                        )
                        y1T.append(t)
                    for h in range(2):
                        ps = psA.tile([P, AGRP], f32, tag="psmm",
                                      name=f"psA2_{g}{h}")
                        nc.tensor.matmul(
                            ps[:], w_sb["xlin_w2"][0][:, h * P:(h + 1) * P],
                            y1T[0][:], start=True, stop=False,
                        )
                        nc.tensor.matmul(
                            ps[:], w_sb["xlin_w2"][1][:, h * P:(h + 1) * P],
                            y1T[1][:], start=False, stop=True,
                        )
                        y2 = pA.tile([P, AGRP], bf16, tag="y2T", name=f"y2T{h}_{g}")
                        if h == 0:
                            nc.scalar.activation(
                                y2[:], ps[:], Relu,
                                bias=b_sb["xlin_b2"][:, h:h + 1]
                            )
                        else:
                            nc.vector.tensor_scalar(
                                out=y2[:], in0=ps[:],
                                scalar1=b_sb["xlin_b2"][:, h:h + 1],
                                scalar2=0.0, op0=ADD, op1=MAX,
                            )
                        nc.vector.tensor_tensor(
                            out=hTw[h][:, gl * AGRP:(gl + 1) * AGRP],
                            in0=xT[h], in1=y2[:], op=ADD,
                        )
                    if gl == 3:
                        for h in range(2):
                            nc.sync.dma_start_transpose(
                                out=h_view[:, h, gw * 16:(gw + 1) * 16, :],
                                in_=hTw[h][:],
                            )

            # ---- edge products + transposes (DVE/SP, ready mid j-loop) ----
            prodT_all = pH.tile([P, NB * 2 * P], bf16, tag="prodT",
                                name="prodT_all")
            prodT_v = prodT_all[:].rearrange("p (b ch e) -> p b ch e",
                                             b=NB, e=P)
            for b in range(NB):
                pt = pE.tile([P, C], bf16, tag=f"prod{b}", name=f"prod{b}")
                nc.vector.tensor_tensor(
                    out=pt[:], in0=exi[b][:], in1=exj[b][:], op=MUL
                )
                nc.sync.dma_start_transpose(out=prodT_v[:, b, :, :], in_=pt[:])

            # ---- stage B j-loop + interleaved stage-C xij layer ----
            with tc.tile_pool(name="psB", bufs=1, space="PSUM") as psB, \
                 tc.tile_pool(name="psC", bufs=3, space="PSUM") as psC, \
                 tc.tile_pool(name="psO", bufs=1, space="PSUM") as psO:
                ps_x = {}
                for hh in range(2):
                    for eh in range(2):
                        ps_x[(hh, eh)] = psB.tile(
                            [P, EL // 2], f32, tag=f"psx{hh}{eh}",
                            name=f"psx{hh}{eh}")

                def mlp_layer(grp, rhs_pair, wname, bname, outtag):
                    W = 512
                    outs = []
                    for h in range(2):
                        ps = psC.tile([P, W], f32, tag="psc",
                                      name=f"psc_{grp}_{outtag}{h}")
                        nc.tensor.matmul(
                            ps[:], w_sb[wname][0][:, h * P:(h + 1) * P],
                            rhs_pair[0], start=True, stop=False,
                        )
                        nc.tensor.matmul(
                            ps[:], w_sb[wname][1][:, h * P:(h + 1) * P],
                            rhs_pair[1], start=False, stop=True,
                        )
                        t = pC.tile([P, W], bf16, tag=f"{outtag}{h}",
                                    name=f"{outtag}{h}_{grp}")
                        nc.scalar.activation(
                            t[:], ps[:], Relu, bias=b_sb[bname][:, h:h + 1]
                        )
                        outs.append(t)
                    return outs

                xijT = {}
                for j in range(NJ):
                    for s in range(2):
                        uj = pU.tile([P, 2 * EL], u16, tag="uj",
                                     name=f"uj{j}_{s}")
                        # single bitvec op: isolate bit j and move it to bit
                        # 14 = the bf16 bit pattern of 2.0 (host pre-halves
                        # xcn_w1 to compensate).  out col = wc*1024+b*128+e;
                        # in iterates (b, wc, e) = T_all's (b, ch=2s+wc, e)
                        ts_kw = dict(
                            out=uj[:].rearrange("p (wc b e) -> p b wc e",
                                                wc=2, e=P),
                            in0=T_view[:, :, 2 * s:2 * s + 2, :],
                            scalar1=1 << j,
                        )
                        if j < 14:
                            nc.vector.tensor_scalar(
                                scalar2=14 - j, op0=ANDB, op1=SHL, **ts_kw)
                        elif j == 14:
                            nc.vector.tensor_scalar(
                                scalar2=None, op0=ANDB, **ts_kw)
                        else:
                            nc.vector.tensor_scalar(
                                scalar2=1, op0=ANDB, op1=SHR, **ts_kw)
                        cj = uj[:].bitcast(bf16)
                        for wc in range(2):
                            kt = s * 32 + 2 * j + wc
                            for hh in range(2):
                                for eh in range(2):
                                    nc.tensor.matmul(
                                        ps_x[(hh, eh)][:],
                                        h_view[:, hh, kt, :],
                                        cj[:, wc * EL + eh * 512:
                                           wc * EL + (eh + 1) * 512],
                                        start=(j == 0 and s == 0 and wc == 0),
                                        stop=(j == NJ - 1 and s == 1
                                              and wc == 1),
                                    )
                    if j == 9:
                        # PE/Act slack inside the j-loop: xij layer for both
                        # 512-edge groups (depends only on prodT)
                        for grp in range(EL // 512):
                            xijT[grp] = mlp_layer(
                                grp,
                                [prodT_v[:, grp * CGRP:(grp + 1) * CGRP, ch, :]
                                 for ch in range(2)],
                                "xij_w", "xij_b", "xijT")

                xcn_sb = pH.tile([P, 2 * EL], bf16, tag="xcn", name="xcn_sb")
                for hh in range(2):
                    for eh in range(2):
                        nc.vector.tensor_copy(
                            xcn_sb[:, hh * EL + eh * 512:
                                   hh * EL + (eh + 1) * 512],
                            ps_x[(hh, eh)][:])

                # ---- stage C xcn-dependent chain ----
                for grp in range(EL // 512):
                    W = 512
                    xcn_rhs = [
                        xcn_sb[:, ch * EL + grp * W:ch * EL + (grp + 1) * W]
                        for ch in range(2)
                    ]
                    u1T = mlp_layer(grp, xcn_rhs, "xcn_w1", "xcn_b1", "u1T")
                    u2T = mlp_layer(grp, [u1T[0][:], u1T[1][:]],
                                    "xcn_w2", "xcn_b2", "u2T")
                    zT = []
                    for h in range(2):
                        zb = pC.tile([P, W], bf16, tag=f"zb{h}",
                                     name=f"zb{h}_{grp}")
                        nc.vector.tensor_tensor(
                            out=zb[:], in0=u2T[h][:],
                            in1=beta_sb.to_broadcast([P, W]), op=MUL,
                        )
                        zt = pC.tile([P, W], bf16, tag=f"zT{h}",
                                     name=f"zT{h}_{grp}")
                        nc.vector.tensor_tensor(
                            out=zt[:], in0=zb[:], in1=xijT[grp][h][:], op=ADD
                        )
                        zT.append(zt)
                    vT = mlp_layer(grp, [zT[0][:], zT[1][:]],
                                   "lin_w1", "lin_b1", "vT")
                    pso = psO.tile([1, W], f32, tag="pso", name=f"pso{grp}")
                    nc.tensor.matmul(
                        pso[:], lw2_sb[0][:], vT[0][:], start=True, stop=False
                    )
                    nc.tensor.matmul(
                        pso[:], lw2_sb[1][:], vT[1][:], start=False, stop=True
                    )
                    nc.scalar.activation(
                        out_row[0:1, grp * W:(grp + 1) * W], pso[:],
                        Ident, bias=lb2_sb[0:1, 0:1],
                    )

            nc.sync.dma_start(out=out_d[:, :], in_=out_row[0:1, :])

    return _split_multi_waits(nc) if split_waits else nc


def kernel(**inputs):
    from concourse.bass_utils import run_bass_kernel_spmd

    if "nc" not in _CACHE:
        _CACHE["nc"] = _build()
    nc = _CACHE["nc"]

    x = np.ascontiguousarray(inputs["x"], dtype=np.float32)
    adj = np.asarray(inputs["adj"])
    # pack adjacency bits into u16 words, k-stride 256 within each 4096-half:
    # adjp[n, s*256+w] bit j = adj[n, s*4096 + j*256 + w]
    A = (adj != 0).astype(np.uint16).reshape(N, 2, 16, 256)
    adjp = np.zeros((N, 2, 256), np.uint16)
    for j in range(16):
        adjp |= A[:, :, j, :] << j
    adjp = np.ascontiguousarray(adjp.reshape(N, 512))
    tar = np.asarray(inputs["tar_ei"]).astype(np.int32)

    def btile(b):
        return np.ascontiguousarray(
            np.asarray(b, dtype=np.float32).reshape(2, P).T)

    WN = ["xlin_w1", "xlin_w2", "xcn_w1", "xcn_w2", "xij_w", "lin_w1"]
    wall = np.zeros((P, 12 * C + 2), np.float32)
    for q, n in enumerate(WN):
        w = np.asarray(inputs[n], dtype=np.float32)  # [C, C]
        if n == "xcn_w1":
            w = w * 0.5  # stage-B cn values are 2.0, not 1.0
        wall[:, (2 * q) * C:(2 * q + 1) * C] = w[:P, :]
        wall[:, (2 * q + 1) * C:(2 * q + 2) * C] = w[P:, :]
    lw2 = np.asarray(inputs["lin_w2"], dtype=np.float32).reshape(C, 1)
    wall[:, 12 * C] = lw2[:P, 0]
    wall[:, 12 * C + 1] = lw2[P:, 0]

    bnames = ["xlin_b1", "xlin_b2", "xcn_b1", "xcn_b2", "xij_b", "lin_b1"]
    ball = np.zeros((P, 2 * len(bnames) + 2), np.float32)
    for q, n in enumerate(bnames):
        ball[:, 2 * q:2 * q + 2] = btile(inputs[n])
    ball[:, 12] = np.asarray(inputs["beta"]).reshape(-1)[0]
    ball[:, 13] = np.asarray(inputs["lin_b2"]).reshape(-1)[0]

    common = {
        "x": x.astype(ml_dtypes.bfloat16),
        "xT": np.ascontiguousarray(x.T).astype(ml_dtypes.bfloat16),
        "adjp": adjp,
        "wall": np.ascontiguousarray(wall).astype(ml_dtypes.bfloat16),
        "ball": np.ascontiguousarray(ball),
    }

    in_maps = []
    for c in range(NCORES):
        m = dict(common)
        m["idx"] = np.ascontiguousarray(tar[:, c * EL:(c + 1) * EL])
        in_maps.append(m)

    res = run_bass_kernel_spmd(
        nc, in_maps, core_ids=list(range(NCORES)), trace=TRACE
    )
    global LAST_RESULT
    LAST_RESULT = res
    out = np.concatenate(
        [res.results[c]["out"].reshape(EL, 1) for c in range(NCORES)], axis=0
    )
    return out.astype(np.float32)
